# revision 4
# baseline (speedup 1.0000x reference)
"""Trainium2 Bass kernel for nn_AutoregressiveGRUWithAttention.

Strategy (data-parallel over batch, 8 cores x 128 batch):
  Everything lives feature-on-partition ("transposed") so the sequential
  GRU recurrence needs zero on-device transposes:
    h+ [65,128] (row64==1 bias row), o+ [65,128] past-prediction carry.
  Gates via tanh only (ACT table set `exp_and_others` has tanh+exp; no
  sigmoid/recip):   sigmoid(v) = 0.5 + 0.5*tanh(v/2)
  -> r/z matmul weights pre-scaled by 0.5, z-gate negated so tanh output
  yields zc = 1-z directly; fused on DVE via scalar_tensor_tensor.
  Encoder valid-length masking folded into the z-gate via a K=1 matmul
  adding -0.5*BIG*(1-valid) to the z psum (zc==0 => h frozen).
  Decoder feedback y is never materialized: gi weights composed with the
  output head on host (Wcomb = Wf^T @ Wih_g^T + bias folds).
  Attention softmax is streamed unnormalized (max|logit| ~ 1.4):
    s += exp(l);  acc += exp(l)*o;  attn = acc * recip_approx_fast(s).
  y output computed transposed-in-PSUM by swapping matmul operands
  (lhsT = o+, rhs = Wf^T+) so PSUM already holds [batch, feat].

PSUM bank plan (per step):  rz bank: r cols 0:128 | z cols 128:256;
  ab bank: A (x-side n gate) | B (0.5*h-side n gate); t3 bank (DVE-written
  tanh input); ly bank: attn logits l | ytr [128,13]. One start=True per
  bank per step (clears the 2KB zero region), others start=False.
"""
import numpy as np

B, L, T, IN, H, OUT = 1024, 64, 128, 13, 64, 13
NCORES, BL = 8, 128
BIG = 60.0

# wh pack column offsets [65 x 461]
_HR, _HZ, _HB, _CR, _CZ, _CA, _WA, _WF = 0, 64, 128, 192, 256, 320, 384, 448
_WH_COLS = 461
# wx pack column offsets [14 x 192]
_XR, _XZ, _XA = 0, 64, 128
_WX_COLS = 192

LAST_EXEC_NS = None
TRACE = False
TRACE_DIR = None


def _prep_weights(Wih, Whh, bih, bhh, Wf, bf, Wa, ba):
    f8 = np.float64
    Wih, Whh, bih, bhh, Wf, bf, Wa, ba = [np.asarray(a, f8) for a in
                                          (Wih, Whh, bih, bhh, Wf, bf, Wa, ba)]
    Wr, Wz, Wn = Wih[0:H], Wih[H:2 * H], Wih[2 * H:3 * H]
    Ur, Uz, Un = Whh[0:H], Whh[H:2 * H], Whh[2 * H:3 * H]
    br_i, bz_i, bn_i = bih[0:H], bih[H:2 * H], bih[2 * H:3 * H]
    br_h, bz_h, bn_h = bhh[0:H], bhh[H:2 * H], bhh[2 * H:3 * H]

    wh = np.zeros((H + 1, _WH_COLS), f8)

    def put_h(col, U, rowbias, scale):
        wh[0:H, col:col + U.shape[0]] = scale * U.T
        wh[H, col:col + U.shape[0]] = scale * rowbias

    put_h(_HR, Ur, br_i + br_h, 0.5)
    put_h(_HZ, Uz, bz_i + bz_h, -0.5)
    put_h(_HB, Un, bn_h, 0.5)                       # B psum = 0.5*(nh + bhh_n)
    # composed decoder input weights:  y @ Wg^T = o @ (Wf^T Wg^T) + bf @ Wg^T
    put_h(_CR, (Wf.T @ Wr.T).T, bf @ Wr.T, 0.5)
    put_h(_CZ, (Wf.T @ Wz.T).T, bf @ Wz.T, -0.5)
    put_h(_CA, (Wf.T @ Wn.T).T, bf @ Wn.T + bn_i, 1.0)
    put_h(_WA, Wa, ba, 1.0)                         # l = o @ Wa^T + ba
    wh[0:H, _WF:_WF + OUT] = Wf.T                   # ytr = o+^T @ WfT+
    wh[H, _WF:_WF + OUT] = bf

    wx = np.zeros((IN + 1, _WX_COLS), f8)
    wx[0:IN, _XR:_XR + H] = 0.5 * Wr.T
    wx[0:IN, _XZ:_XZ + H] = -0.5 * Wz.T
    wx[0:IN, _XA:_XA + H] = Wn.T
    wx[IN, _XA:_XA + H] = bn_i                      # A psum = inn + bih_n

    mrow = np.full((1, H), -0.5 * BIG, f8)
    return (np.ascontiguousarray(wh, np.float32),
            np.ascontiguousarray(wx, np.float32),
            np.ascontiguousarray(mrow, np.float32))


def _prep_core(x_core, len_core, l_steps=L):
    x_core = np.asarray(x_core, np.float32)
    xT = np.zeros((IN + 1, l_steps, BL), np.float32)
    xT[0:IN] = np.transpose(x_core[:, 0:l_steps, :], (2, 1, 0))
    xT[IN] = 1.0
    valid = (np.arange(l_steps)[:, None] < np.asarray(len_core)[None, :])
    invm = np.ascontiguousarray(
        (1.0 - valid.astype(np.float32)).reshape(1, l_steps * BL))
    m63 = valid[l_steps - 1].astype(np.float32)
    m63bc = np.ascontiguousarray(np.broadcast_to(m63, (H, BL)), np.float32)
    return xT.reshape(IN + 1, l_steps * BL).copy(), invm, m63bc


def build_nc(l_steps=L, t_steps=T, compile=True):
    import concourse.bass as bass
    import concourse.bacc as bacc
    import concourse.tile as tile
    from concourse import mybir
    from contextlib import ExitStack

    f32 = mybir.dt.float32
    AF = mybir.ActivationFunctionType
    OP = mybir.AluOpType

    nc = bacc.Bacc("TRN2", target_bir_lowering=False, debug=False,
                   num_devices=NCORES)
    d_xT = nc.declare_dram_parameter("xT", [IN + 1, l_steps * BL], f32, isOutput=False)
    d_invm = nc.declare_dram_parameter("invm", [1, l_steps * BL], f32, isOutput=False)
    d_m63 = nc.declare_dram_parameter("m63", [H, BL], f32, isOutput=False)
    d_wh = nc.declare_dram_parameter("wh", [H + 1, _WH_COLS], f32, isOutput=False)
    d_wx = nc.declare_dram_parameter("wx", [IN + 1, _WX_COLS], f32, isOutput=False)
    d_mrow = nc.declare_dram_parameter("mrow", [1, H], f32, isOutput=False)
    d_out = nc.declare_dram_parameter("out", [BL, t_steps * OUT], f32, isOutput=True)

    with tile.TileContext(nc) as tc, ExitStack() as ctx:
        const = ctx.enter_context(tc.tile_pool(name="const", bufs=1))
        temps = ctx.enter_context(tc.tile_pool(name="temps", bufs=3))
        p_rz = ctx.enter_context(tc.tile_pool(name="p_rz", bufs=2, space="PSUM"))
        p_ab = ctx.enter_context(tc.tile_pool(name="p_ab", bufs=2, space="PSUM"))
        p_t3 = ctx.enter_context(tc.tile_pool(name="p_t3", bufs=2, space="PSUM"))
        p_ly = ctx.enter_context(tc.tile_pool(name="p_ly", bufs=2, space="PSUM"))

        # ---- persistent sbuf ----
        xT = const.tile([IN + 1, l_steps * BL], f32)
        invm = const.tile([1, l_steps * BL], f32)
        m63 = const.tile([H, BL], f32)
        wh = const.tile([H + 1, _WH_COLS], f32)
        wx = const.tile([IN + 1, _WX_COLS], f32)
        mrow = const.tile([1, H], f32)
        hS = const.tile([H + 1, BL], f32)      # h+ state
        oS = const.tile([H + 1, BL], f32)      # o+ carry
        sS = const.tile([H, BL], f32)          # attn denominator
        accS = const.tile([H, BL], f32)        # attn numerator
        out_sb = const.tile([BL, t_steps * OUT], f32)

        nc.sync.dma_start(out=xT, in_=d_xT[:])
        nc.sync.dma_start(out=invm, in_=d_invm[:])
        nc.sync.dma_start(out=m63, in_=d_m63[:])
        nc.sync.dma_start(out=wh, in_=d_wh[:])
        nc.sync.dma_start(out=wx, in_=d_wx[:])
        nc.sync.dma_start(out=mrow, in_=d_mrow[:])

        nc.vector.memset(hS[0:H, :], 0.0)
        nc.vector.memset(hS[H:H + 1, :], 1.0)
        nc.vector.memset(oS[H:H + 1, :], 1.0)
        nc.vector.memset(sS, 0.0)
        nc.vector.memset(accS, 0.0)

        h64 = hS[0:H, :]
        o64 = oS[0:H, :]

        def gate_front(rhs_in, xr, xz, xa, wsrc, mask_rhs):
            """Emit gate matmuls + tanh + n-path front; returns (trz, d, t3ps)."""
            rz = p_rz.tile([H, 256], f32, tag="rz")
            nc.tensor.matmul(rz[:, 0:128], wh[:, _HR:_HR + H], hS[:],
                             start=True, stop=False)
            nc.tensor.matmul(rz[:, 0:128], wsrc[:, xr:xr + H], rhs_in,
                             start=False, stop=False)
            nc.tensor.matmul(rz[:, 128:256], wh[:, _HZ:_HZ + H], hS[:],
                             start=False, stop=False)
            last_z = mask_rhs is None
            nc.tensor.matmul(rz[:, 128:256], wsrc[:, xz:xz + H], rhs_in,
                             start=False, stop=last_z)
            if mask_rhs is not None:
                nc.tensor.matmul(rz[:, 128:256], mrow[:], mask_rhs,
                                 start=False, stop=True)
            ab = p_ab.tile([H, 256], f32, tag="ab")
            nc.tensor.matmul(ab[:, 0:128], wsrc[:, xa:xa + H], rhs_in,
                             start=True, stop=False)
            nc.tensor.matmul(ab[:, 128:256], wh[:, _HB:_HB + H], hS[:],
                             start=False, stop=True)

            trz = temps.tile([H, 256], f32, tag="trz")
            nc.scalar.activation(out=trz, in_=rz[:], func=AF.Tanh)
            # t2 = (tanh_r + 1) * B   (B pre-halved)  -> t3 = t2 + A
            t2 = temps.tile([H, BL], f32, tag="t2")
            nc.vector.scalar_tensor_tensor(out=t2, in0=trz[:, 0:128], scalar=1.0,
                                           in1=ab[:, 128:256], op0=OP.add,
                                           op1=OP.mult)
            t3 = p_t3.tile([H, BL], f32, tag="t3")
            nc.vector.tensor_add(out=t3, in0=t2, in1=ab[:, 0:128])
            n = temps.tile([H, BL], f32, tag="n")
            nc.scalar.activation(out=n, in_=t3[:], func=AF.Tanh)
            d = temps.tile([H, BL], f32, tag="d")
            nc.vector.tensor_sub(out=d, in0=n, in1=h64)
            tzd = temps.tile([H, BL], f32, tag="tzd")
            nc.vector.scalar_tensor_tensor(out=tzd, in0=trz[:, 128:256], scalar=1.0,
                                           in1=d, op0=OP.add, op1=OP.mult)
            return tzd

        # ================= encoder =================
        for t in range(l_steps):
            rhs_x = xT[:, t * BL:(t + 1) * BL]
            mask_rhs = invm[:, t * BL:(t + 1) * BL] if t < l_steps - 1 else None
            tzd = gate_front(rhs_x, _XR, _XZ, _XA, wx, mask_rhs)
            if t < l_steps - 1:
                # h += 0.5 * tzd   (z-gate already carries the freeze mask)
                nc.vector.scalar_tensor_tensor(out=h64, in0=tzd, scalar=0.5,
                                               in1=h64, op0=OP.mult, op1=OP.add)
            else:
                # last step: unmasked gates; compute hn, masked h, masked out
                nc.vector.scalar_tensor_tensor(out=o64, in0=tzd, scalar=0.5,
                                               in1=h64, op0=OP.mult, op1=OP.add)
                u = temps.tile([H, BL], f32, tag="u")
                nc.vector.tensor_mul(out=u, in0=m63, in1=tzd)
                nc.vector.scalar_tensor_tensor(out=h64, in0=u, scalar=0.5,
                                               in1=h64, op0=OP.mult, op1=OP.add)
                nc.vector.tensor_mul(out=o64, in0=o64, in1=m63)

        # ================= decoder =================
        for t in range(t_steps):
            if t > 0:
                rec = temps.tile([H, BL], f32, tag="rec")
                nc.vector.reciprocal_approx_fast(out=rec, in_=sS[:])
                attnt = temps.tile([H, BL], f32, tag="attnt")
                nc.vector.tensor_mul(out=attnt, in0=accS[:], in1=rec)
            else:
                attnt = None
            tzd = gate_front(oS[:], _CR, _CZ, _CA, wh, None)
            nc.vector.scalar_tensor_tensor(out=h64, in0=tzd, scalar=0.5,
                                           in1=h64, op0=OP.mult, op1=OP.add)
            if attnt is None:
                nc.vector.tensor_copy(out=o64, in_=h64)
            else:
                nc.vector.tensor_add(out=o64, in0=h64, in1=attnt)
            # attention logits + transposed output, one psum bank
            ly = p_ly.tile([BL, 141], f32, tag="ly")
            nc.tensor.matmul(ly[0:H, 0:128], wh[:, _WA:_WA + H], oS[:],
                             start=True, stop=True)
            nc.tensor.matmul(ly[0:BL, 128:141], oS[:], wh[:, _WF:_WF + OUT],
                             start=True, stop=True)
            e = temps.tile([H, BL], f32, tag="e")
            nc.scalar.activation(out=e, in_=ly[0:H, 0:128], func=AF.Exp)
            nc.gpsimd.tensor_add(out=sS, in0=sS[:], in1=e)
            eo = temps.tile([H, BL], f32, tag="eo")
            nc.gpsimd.tensor_mul(out=eo, in0=e, in1=o64)
            nc.gpsimd.tensor_add(out=accS, in0=accS[:], in1=eo)
            nc.scalar.copy(out=out_sb[:, t * OUT:(t + 1) * OUT],
                           in_=ly[0:BL, 128:141])

        nc.sync.dma_start(out=d_out[:], in_=out_sb)
    if compile:
        nc.compile()
    return nc


def _make_in_maps(inputs, l_steps=L, t_steps=T):
    x = np.asarray(inputs["x"], np.float32)
    lengths = np.asarray(inputs["lengths"])
    wh, wx, mrow = _prep_weights(inputs["Wih"], inputs["Whh"], inputs["bih"],
                                 inputs["bhh"], inputs["Wf"], inputs["bf"],
                                 inputs["Wa"], inputs["ba"])
    in_maps = []
    for c in range(NCORES):
        sl = slice(c * BL, (c + 1) * BL)
        xT, invm, m63 = _prep_core(x[sl], lengths[sl], l_steps)
        in_maps.append(dict(xT=xT, invm=invm, m63=m63, wh=wh, wx=wx, mrow=mrow))
    return in_maps


def kernel(**inputs):
    global LAST_EXEC_NS, TRACE_DIR
    from concourse.bass_utils import run_bass_kernel_spmd
    t_steps = int(inputs.get("output_length", T))
    assert t_steps == T, f"hardcoded for output_length={T}, got {t_steps}"
    nc = build_nc()
    in_maps = _make_in_maps(inputs)
    kw = {}
    if TRACE:
        import tempfile
        TRACE_DIR = tempfile.mkdtemp(prefix="bass_trace_")
        kw = dict(trace=True, tmpdir=TRACE_DIR)
    res = run_bass_kernel_spmd(nc, in_maps, list(range(NCORES)), **kw)
    LAST_EXEC_NS = res.exec_time_ns
    outs = [np.asarray(res.results[c]["out"]).reshape(BL, T, OUT)
            for c in range(NCORES)]
    return np.concatenate(outs, axis=0)


# revision 7
# speedup vs baseline: 1.3152x; 1.3152x over previous
"""Trainium2 Bass kernel for nn_AutoregressiveGRUWithAttention.

Strategy (data-parallel over batch, 8 cores x 128 batch):
  Feature-on-partition ("transposed") layout -> zero on-device transposes:
    h+ [65,128] fp32 state (row64==1 bias row) + bf16 shadow for matmuls
    o+ [65,128] fp32 past-prediction carry + bf16 shadow.
  Gate matmuls in bf16 (fp32 matmuls are 2 half-rate passes on TRN2 ~8x cost).
  Gates via tanh only (ACT set `exp_and_others`): sigmoid(v)=0.5+0.5*tanh(v/2)
  with 0.5 pre-folded into r/z weights, z negated so tanh yields zc=1-z.
  n-gate: psum_A(=x-side) accumulates r*B via a PE identity-matmul of the
  bf16 DVE product t2=(tanh_r+1)*B -> tanh_n reads the finished psum.
  Encoder valid-length mask folded into z-gate via K=1 matmul of
  -0.5*BIG*(1-valid) (zc==0 => h frozen); last step handled explicitly.
  Decoder feedback y never materialized: composed weights Wcomb=Wf^T@Wg^T.
  Attention softmax streamed unnormalized (max|logit|~1.4):
    sacc[64,256] = [s | acc];  eeo[64,256] = [exp(l) | exp(l)*o] (ACT+GpSimd)
    sacc += eeo (one fused GpSimd add);  attn = acc * recip_approx_fast(s).
  y output computed transposed-in-PSUM by swapping matmul operands
  (lhsT = o+ fp32, rhs = Wf^T+ fp32) so PSUM holds [batch, feat]; fp32 head.

PSUM banks: rz[64,256] r|z, ab[64,256] n-input|B, ly[128,141] l|ytr, bufs=2.
"""
import numpy as np
import ml_dtypes

B, L, T, IN, H, OUT = 1024, 64, 128, 13, 64, 13
NCORES, BL = 8, 128
BIG = 60.0
BF16 = ml_dtypes.bfloat16

# wh pack column offsets [65 x 448] (bf16)
_HR, _HZ, _HB, _CR, _CZ, _CA, _WA = 0, 64, 128, 192, 256, 320, 384
_WH_COLS = 448
# wx pack column offsets [14 x 192] (bf16)
_XR, _XZ, _XA = 0, 64, 128
_WX_COLS = 192

LAST_EXEC_NS = None
TRACE = False
TRACE_DIR = None


def _prep_weights(Wih, Whh, bih, bhh, Wf, bf, Wa, ba):
    f8 = np.float64
    Wih, Whh, bih, bhh, Wf, bf, Wa, ba = [np.asarray(a, f8) for a in
                                          (Wih, Whh, bih, bhh, Wf, bf, Wa, ba)]
    Wr, Wz, Wn = Wih[0:H], Wih[H:2 * H], Wih[2 * H:3 * H]
    Ur, Uz, Un = Whh[0:H], Whh[H:2 * H], Whh[2 * H:3 * H]
    br_i, bz_i, bn_i = bih[0:H], bih[H:2 * H], bih[2 * H:3 * H]
    br_h, bz_h, bn_h = bhh[0:H], bhh[H:2 * H], bhh[2 * H:3 * H]

    wh = np.zeros((H + 1, _WH_COLS), f8)

    def put_h(col, U, rowbias, scale):
        wh[0:H, col:col + U.shape[0]] = scale * U.T
        wh[H, col:col + U.shape[0]] = scale * rowbias

    put_h(_HR, Ur, br_i + br_h, 0.5)
    put_h(_HZ, Uz, bz_i + bz_h, -0.5)
    put_h(_HB, Un, bn_h, 0.5)                       # B psum = 0.5*(nh + bhh_n)
    put_h(_CR, (Wf.T @ Wr.T).T, bf @ Wr.T, 0.5)
    put_h(_CZ, (Wf.T @ Wz.T).T, bf @ Wz.T, -0.5)
    put_h(_CA, (Wf.T @ Wn.T).T, bf @ Wn.T + bn_i, 1.0)
    put_h(_WA, Wa, ba, 1.0)                         # l = o @ Wa^T + ba

    wf = np.zeros((H + 1, OUT), f8)                 # fp32 output head
    wf[0:H] = Wf.T
    wf[H] = bf

    wx = np.zeros((IN + 1, _WX_COLS), f8)
    wx[0:IN, _XR:_XR + H] = 0.5 * Wr.T
    wx[0:IN, _XZ:_XZ + H] = -0.5 * Wz.T
    wx[0:IN, _XA:_XA + H] = Wn.T
    wx[IN, _XA:_XA + H] = bn_i                      # A psum = inn + bih_n

    mrow = np.full((1, H), -0.5 * BIG, f8)
    ident = np.eye(H, dtype=f8)
    return dict(
        wh=np.ascontiguousarray(wh, BF16),
        wf=np.ascontiguousarray(wf, np.float32),
        wx=np.ascontiguousarray(wx, BF16),
        mrow=np.ascontiguousarray(mrow, BF16),
        ident=np.ascontiguousarray(ident, BF16),
    )


def _prep_core(x_core, len_core, l_steps=L):
    x_core = np.asarray(x_core, np.float32)
    xT = np.zeros((IN + 1, l_steps, BL), np.float32)
    xT[0:IN] = np.transpose(x_core[:, 0:l_steps, :], (2, 1, 0))
    xT[IN] = 1.0
    valid = (np.arange(l_steps)[:, None] < np.asarray(len_core)[None, :])
    invm = (1.0 - valid.astype(np.float32)).reshape(1, l_steps * BL)
    m63 = valid[l_steps - 1].astype(np.float32)
    m63bc = np.ascontiguousarray(np.broadcast_to(m63, (H, BL)), np.float32)
    return (np.ascontiguousarray(xT.reshape(IN + 1, l_steps * BL), BF16),
            np.ascontiguousarray(invm, BF16), m63bc)


def build_nc(l_steps=L, t_steps=T, compile=True):
    import concourse.bacc as bacc
    import concourse.tile as tile
    from concourse import mybir
    from contextlib import ExitStack

    f32 = mybir.dt.float32
    bf = mybir.dt.bfloat16
    AF = mybir.ActivationFunctionType
    OP = mybir.AluOpType

    nc = bacc.Bacc("TRN2", target_bir_lowering=False, debug=False,
                   num_devices=NCORES)
    d_xT = nc.declare_dram_parameter("xT", [IN + 1, l_steps * BL], bf, isOutput=False)
    d_invm = nc.declare_dram_parameter("invm", [1, l_steps * BL], bf, isOutput=False)
    d_m63 = nc.declare_dram_parameter("m63", [H, BL], f32, isOutput=False)
    d_wh = nc.declare_dram_parameter("wh", [H + 1, _WH_COLS], bf, isOutput=False)
    d_wf = nc.declare_dram_parameter("wf", [H + 1, OUT], f32, isOutput=False)
    d_wx = nc.declare_dram_parameter("wx", [IN + 1, _WX_COLS], bf, isOutput=False)
    d_mrow = nc.declare_dram_parameter("mrow", [1, H], bf, isOutput=False)
    d_ident = nc.declare_dram_parameter("ident", [H, H], bf, isOutput=False)
    d_out = nc.declare_dram_parameter("out", [BL, t_steps * OUT], f32, isOutput=True)

    with tile.TileContext(nc) as tc, ExitStack() as ctx:
        const = ctx.enter_context(tc.tile_pool(name="const", bufs=1))
        temps = ctx.enter_context(tc.tile_pool(name="temps", bufs=3))
        p_rz = ctx.enter_context(tc.tile_pool(name="p_rz", bufs=2, space="PSUM"))
        p_a = ctx.enter_context(tc.tile_pool(name="p_a", bufs=2, space="PSUM"))
        p_b = ctx.enter_context(tc.tile_pool(name="p_b", bufs=2, space="PSUM"))
        p_ly = ctx.enter_context(tc.tile_pool(name="p_ly", bufs=2, space="PSUM"))

        # ---- persistent sbuf ----
        xT = const.tile([IN + 1, l_steps * BL], bf)
        invm = const.tile([1, l_steps * BL], bf)
        m63 = const.tile([H, BL], f32)
        wh = const.tile([H + 1, _WH_COLS], bf)
        wf = const.tile([H + 1, OUT], f32)
        wx = const.tile([IN + 1, _WX_COLS], bf)
        mrow = const.tile([1, H], bf)
        ident = const.tile([H, H], bf)
        hS = const.tile([H + 1, BL], f32)      # h+ fp32 state
        hB = const.tile([H + 1, BL], bf)       # h+ bf16 shadow
        oS = const.tile([H + 1, BL], f32)      # o+ fp32 carry
        oB = const.tile([H + 1, BL], bf)       # o+ bf16 shadow
        sacc = const.tile([H, 2 * BL], f32)    # [s | acc]
        out_sb = const.tile([BL, t_steps * OUT], f32)

        for dst, src in ((xT, d_xT), (invm, d_invm), (m63, d_m63), (wh, d_wh),
                         (wf, d_wf), (wx, d_wx), (mrow, d_mrow), (ident, d_ident)):
            nc.sync.dma_start(out=dst, in_=src[:])

        nc.vector.memset(hS[0:H, :], 0.0)
        nc.vector.memset(hS[H:H + 1, :], 1.0)
        nc.vector.memset(hB[0:H, :], 0.0)
        nc.vector.memset(hB[H:H + 1, :], 1.0)
        nc.vector.memset(oB[H:H + 1, :], 1.0)
        nc.vector.memset(oS[H:H + 1, :], 1.0)
        nc.vector.memset(sacc, 0.0)

        h64 = hS[0:H, :]
        o64 = oS[0:H, :]

        def gate_front(rhs_bf, xr, xz, xa, wsrc, mask_rhs):
            """Gate matmuls + tanh pipeline; returns (d_unset, tzd) via temps.
            Emits: rz psums, ab psums (+identity-matmul accumulate of t2),
            tanh_r/z/n, t2, d, tzd."""
            rz = p_rz.tile([H, 256], f32, tag="rz")
            nc.tensor.matmul(rz[:, 0:128], wh[:, _HR:_HR + H], hB[:],
                             start=True, stop=False)
            nc.tensor.matmul(rz[:, 0:128], wsrc[:, xr:xr + H], rhs_bf,
                             start=False, stop=False)
            nc.tensor.matmul(rz[:, 128:256], wh[:, _HZ:_HZ + H], hB[:],
                             start=False, stop=False)
            last_z = mask_rhs is None
            nc.tensor.matmul(rz[:, 128:256], wsrc[:, xz:xz + H], rhs_bf,
                             start=False, stop=last_z)
            if mask_rhs is not None:
                nc.tensor.matmul(rz[:, 128:256], mrow[:], mask_rhs,
                                 start=False, stop=True)
            pa = p_a.tile([H, BL], f32, tag="pa")
            nc.tensor.matmul(pa[:], wsrc[:, xa:xa + H], rhs_bf,
                             start=True, stop=False)
            pb = p_b.tile([H, BL], f32, tag="pb")
            nc.tensor.matmul(pb[:], wh[:, _HB:_HB + H], hB[:],
                             start=True, stop=True)

            tr = temps.tile([H, BL], bf, tag="tr")
            nc.scalar.activation(out=tr, in_=rz[:, 0:128], func=AF.Tanh)
            tz = temps.tile([H, BL], bf, tag="tz")
            nc.scalar.activation(out=tz, in_=rz[:, 128:256], func=AF.Tanh)
            # t2 = (tanh_r + 1) * B  (B pre-halved), bf16 for the PE accumulate
            t2 = temps.tile([H, BL], bf, tag="t2")
            nc.vector.scalar_tensor_tensor(out=t2, in0=tr, scalar=1.0,
                                           in1=pb[:], op0=OP.add,
                                           op1=OP.mult)
            nc.tensor.matmul(pa[:], ident[:], t2,
                             start=False, stop=True)
            n = temps.tile([H, BL], f32, tag="n")
            nc.scalar.activation(out=n, in_=pa[:], func=AF.Tanh)
            d = temps.tile([H, BL], f32, tag="d")
            nc.vector.tensor_sub(out=d, in0=n, in1=h64)
            tzd = temps.tile([H, BL], f32, tag="tzd")
            nc.vector.scalar_tensor_tensor(out=tzd, in0=tz, scalar=1.0,
                                           in1=d, op0=OP.add, op1=OP.mult)
            return tzd

        # ================= encoder =================
        for t in range(l_steps):
            rhs_x = xT[:, t * BL:(t + 1) * BL]
            mask_rhs = invm[:, t * BL:(t + 1) * BL] if t < l_steps - 1 else None
            tzd = gate_front(rhs_x, _XR, _XZ, _XA, wx, mask_rhs)
            if t < l_steps - 1:
                nc.vector.scalar_tensor_tensor(out=h64, in0=tzd, scalar=0.5,
                                               in1=h64, op0=OP.mult, op1=OP.add)
            else:
                nc.vector.scalar_tensor_tensor(out=o64, in0=tzd, scalar=0.5,
                                               in1=h64, op0=OP.mult, op1=OP.add)
                u = temps.tile([H, BL], f32, tag="u")
                nc.vector.tensor_mul(out=u, in0=m63, in1=tzd)
                nc.vector.scalar_tensor_tensor(out=h64, in0=u, scalar=0.5,
                                               in1=h64, op0=OP.mult, op1=OP.add)
                nc.vector.tensor_mul(out=o64, in0=o64, in1=m63)
                nc.vector.tensor_copy(out=oB[0:H, :], in_=o64)
            nc.vector.tensor_copy(out=hB[0:H, :], in_=h64)

        # ================= decoder =================
        for t in range(t_steps):
            if t > 0:
                rec = temps.tile([H, BL], f32, tag="rec")
                nc.vector.reciprocal_approx_fast(out=rec, in_=sacc[:, 0:BL])
                attnt = temps.tile([H, BL], f32, tag="attnt")
                nc.vector.tensor_mul(out=attnt, in0=sacc[:, BL:2 * BL], in1=rec)
            else:
                attnt = None
            tzd = gate_front(oB[:], _CR, _CZ, _CA, wh, None)
            nc.vector.scalar_tensor_tensor(out=h64, in0=tzd, scalar=0.5,
                                           in1=h64, op0=OP.mult, op1=OP.add)
            nc.vector.tensor_copy(out=hB[0:H, :], in_=h64)
            if attnt is None:
                nc.vector.tensor_copy(out=o64, in_=h64)
            else:
                nc.vector.tensor_add(out=o64, in0=h64, in1=attnt)
            nc.vector.tensor_copy(out=oB[0:H, :], in_=o64)
            # attention logits (bf16) + transposed fp32 output head, one bank
            ly = p_ly.tile([BL, 141], f32, tag="ly")
            nc.tensor.matmul(ly[0:H, 0:128], wh[:, _WA:_WA + H], oB[:],
                             start=True, stop=True)
            nc.tensor.matmul(ly[0:BL, 128:141], oS[:], wf[:],
                             start=True, stop=True)
            eeo = temps.tile([H, 2 * BL], f32, tag="eeo")
            nc.scalar.activation(out=eeo[:, 0:BL], in_=ly[0:H, 0:128], func=AF.Exp)
            nc.gpsimd.tensor_mul(out=eeo[:, BL:2 * BL], in0=eeo[:, 0:BL], in1=o64)
            nc.gpsimd.tensor_add(out=sacc, in0=sacc[:], in1=eeo)
            nc.scalar.copy(out=out_sb[:, t * OUT:(t + 1) * OUT],
                           in_=ly[0:BL, 128:141])

        nc.sync.dma_start(out=d_out[:], in_=out_sb)
    if compile:
        nc.compile()
    return nc


def _make_in_maps(inputs, l_steps=L, t_steps=T):
    x = np.asarray(inputs["x"], np.float32)
    lengths = np.asarray(inputs["lengths"])
    w = _prep_weights(inputs["Wih"], inputs["Whh"], inputs["bih"],
                      inputs["bhh"], inputs["Wf"], inputs["bf"],
                      inputs["Wa"], inputs["ba"])
    in_maps = []
    for c in range(NCORES):
        sl = slice(c * BL, (c + 1) * BL)
        xT, invm, m63 = _prep_core(x[sl], lengths[sl], l_steps)
        in_maps.append(dict(xT=xT, invm=invm, m63=m63, **w))
    return in_maps


def kernel(**inputs):
    global LAST_EXEC_NS, TRACE_DIR
    from concourse.bass_utils import run_bass_kernel_spmd
    t_steps = int(inputs.get("output_length", T))
    assert t_steps == T, f"hardcoded for output_length={T}, got {t_steps}"
    nc = build_nc()
    in_maps = _make_in_maps(inputs)
    kw = {}
    if TRACE:
        import tempfile
        TRACE_DIR = tempfile.mkdtemp(prefix="bass_trace_")
        kw = dict(trace=True, tmpdir=TRACE_DIR)
    res = run_bass_kernel_spmd(nc, in_maps, list(range(NCORES)), **kw)
    LAST_EXEC_NS = res.exec_time_ns
    outs = [np.asarray(res.results[c]["out"]).reshape(BL, T, OUT)
            for c in range(NCORES)]
    return np.concatenate(outs, axis=0)


# revision 9
# speedup vs baseline: 1.5613x; 1.1872x over previous
"""Trainium2 Bass kernel for nn_AutoregressiveGRUWithAttention.

Strategy (data-parallel over batch, 8 cores x 128 batch):
  Feature-on-partition ("transposed") layout -> zero on-device transposes.
  States kept bf16-only: hB [65,128] (row64==1), oB [65,128], attnB [65,128]
  (row64==0). All matmuls bf16 (fp32 mm on TRN2 = 2 half-rate passes).
  Gates via tanh only (ACT set `exp_and_others`): sigmoid(v)=0.5+0.5tanh(v/2),
  0.5 folded into r/z weights, z negated so tanh gives zc=1-z.
  Decoder input o(t-1)=h(t-1)+attn(t-1) is never materialized for the gate
  matmuls: folded weights FR=HR+CR etc. act on h, and CR/CZ/CA act on the
  (early-available) attnB(t-1) -> only ONE matmul on the recurrence chain.
  n-gate: t3 = tanh_r*B' + A2 with A2=(CA+HB)-fold; t3 written back into B's
  psum bank by DVE; h-mix in P-form: h' = 0.5(tz+1)n + P, P=0.5h(1-tz)
  computed off-chain early.
  Encoder valid-length mask folded into z-gate via K=1 matmul of
  -0.5*BIG*(1-valid); last encoder step handled explicitly (d-form).
  Attention softmax streamed unnormalized (max|logit| ~ 1.4):
    sacc[64,256]=[s|acc]; eeo[64,256]=[exp(l)|exp(l)*o]; one fused GpSimd add;
    attn = acc * recip_approx_fast(s).
  y head: transposed-in-PSUM via swapped operands (lhsT=oB, rhs=WF bf16).

PSUM banks: rz[64,256] r|z, pa[64,128], pb[64,128] (B then t3), ly[128,141].
"""
import numpy as np
import ml_dtypes

B, L, T, IN, H, OUT = 1024, 64, 128, 13, 64, 13
NCORES, BL = 8, 128
BIG = 60.0
BF16 = ml_dtypes.bfloat16

# wh pack column offsets [65 x 653] (bf16)
_HR, _HZ, _HB, _CR, _CZ, _CA, _WA = 0, 64, 128, 192, 256, 320, 384
_FR, _FZ, _FA2, _WF = 448, 512, 576, 640
_WH_COLS = 653
# wx pack column offsets [14 x 192] (bf16)
_XR, _XZ, _XA = 0, 64, 128
_WX_COLS = 192

LAST_EXEC_NS = None
TRACE = False
TRACE_DIR = None
WARM_DUMMIES = 0


def _prep_weights(Wih, Whh, bih, bhh, Wf, bf, Wa, ba):
    f8 = np.float64
    Wih, Whh, bih, bhh, Wf, bf, Wa, ba = [np.asarray(a, f8) for a in
                                          (Wih, Whh, bih, bhh, Wf, bf, Wa, ba)]
    Wr, Wz, Wn = Wih[0:H], Wih[H:2 * H], Wih[2 * H:3 * H]
    Ur, Uz, Un = Whh[0:H], Whh[H:2 * H], Whh[2 * H:3 * H]
    br_i, bz_i, bn_i = bih[0:H], bih[H:2 * H], bih[2 * H:3 * H]
    br_h, bz_h, bn_h = bhh[0:H], bhh[H:2 * H], bhh[2 * H:3 * H]

    def blk(rows, rowbias, scale):
        m = np.zeros((H + 1, rows.shape[0]), f8)
        m[0:H] = scale * rows.T
        m[H] = scale * rowbias
        return m

    HRm = blk(Ur, br_i + br_h, 0.5)
    HZm = blk(Uz, bz_i + bz_h, -0.5)
    HBm = blk(Un, bn_h, 0.5)                        # B' = 0.5*(nh + bhh_n)
    CRm = blk((Wf.T @ Wr.T).T, bf @ Wr.T, 0.5)
    CZm = blk((Wf.T @ Wz.T).T, bf @ Wz.T, -0.5)
    CAm = blk((Wf.T @ Wn.T).T, bf @ Wn.T + bn_i, 1.0)
    WAm = blk(Wa, ba, 1.0)

    wh = np.zeros((H + 1, _WH_COLS), f8)
    for col, m in ((_HR, HRm), (_HZ, HZm), (_HB, HBm), (_CR, CRm), (_CZ, CZm),
                   (_CA, CAm), (_WA, WAm), (_FR, HRm + CRm), (_FZ, HZm + CZm),
                   (_FA2, CAm + HBm)):
        wh[:, col:col + H] = m
    wh[0:H, _WF:_WF + OUT] = Wf.T
    wh[H, _WF:_WF + OUT] = bf

    wx = np.zeros((IN + 1, _WX_COLS), f8)
    wx[0:IN, _XR:_XR + H] = 0.5 * Wr.T
    wx[0:IN, _XZ:_XZ + H] = -0.5 * Wz.T
    wx[0:IN, _XA:_XA + H] = Wn.T
    wx[IN, _XA:_XA + H] = bn_i

    mrow = np.full((1, H), -0.5 * BIG, f8)
    return dict(
        wh=np.ascontiguousarray(wh, BF16),
        wx=np.ascontiguousarray(wx, BF16),
        mrow=np.ascontiguousarray(mrow, BF16),
    )


def _prep_core(x_core, len_core, l_steps=L):
    x_core = np.asarray(x_core, np.float32)
    xT = np.zeros((IN + 1, l_steps, BL), np.float32)
    xT[0:IN] = np.transpose(x_core[:, 0:l_steps, :], (2, 1, 0))
    xT[IN] = 1.0
    valid = (np.arange(l_steps)[:, None] < np.asarray(len_core)[None, :])
    invm = (1.0 - valid.astype(np.float32)).reshape(1, l_steps * BL)
    m63 = valid[l_steps - 1].astype(np.float32)
    m63bc = np.ascontiguousarray(np.broadcast_to(m63, (H, BL)), np.float32)
    return (np.ascontiguousarray(xT.reshape(IN + 1, l_steps * BL), BF16),
            np.ascontiguousarray(invm, BF16), m63bc)


def build_nc(l_steps=L, t_steps=T, compile=True):
    import concourse.bacc as bacc
    import concourse.tile as tile
    from concourse import mybir
    from contextlib import ExitStack

    f32 = mybir.dt.float32
    bf = mybir.dt.bfloat16
    AF = mybir.ActivationFunctionType
    OP = mybir.AluOpType

    nc = bacc.Bacc("TRN2", target_bir_lowering=False, debug=False,
                   num_devices=NCORES)
    d_xT = nc.declare_dram_parameter("xT", [IN + 1, l_steps * BL], bf, isOutput=False)
    d_invm = nc.declare_dram_parameter("invm", [1, l_steps * BL], bf, isOutput=False)
    d_m63 = nc.declare_dram_parameter("m63", [H, BL], f32, isOutput=False)
    d_wh = nc.declare_dram_parameter("wh", [H + 1, _WH_COLS], bf, isOutput=False)
    d_wx = nc.declare_dram_parameter("wx", [IN + 1, _WX_COLS], bf, isOutput=False)
    d_mrow = nc.declare_dram_parameter("mrow", [1, H], bf, isOutput=False)
    d_out = nc.declare_dram_parameter("out", [BL, t_steps * OUT], f32, isOutput=True)

    with tile.TileContext(nc) as tc, ExitStack() as ctx:
        const = ctx.enter_context(tc.tile_pool(name="const", bufs=1))
        temps = ctx.enter_context(tc.tile_pool(name="temps", bufs=3))
        p_rz = ctx.enter_context(tc.tile_pool(name="p_rz", bufs=2, space="PSUM"))
        p_a = ctx.enter_context(tc.tile_pool(name="p_a", bufs=2, space="PSUM"))
        p_b = ctx.enter_context(tc.tile_pool(name="p_b", bufs=2, space="PSUM"))
        p_ly = ctx.enter_context(tc.tile_pool(name="p_ly", bufs=2, space="PSUM"))

        xT = const.tile([IN + 1, l_steps * BL], bf)
        invm = const.tile([1, l_steps * BL], bf)
        m63 = const.tile([H, BL], f32)
        wh = const.tile([H + 1, _WH_COLS], bf)
        wx = const.tile([IN + 1, _WX_COLS], bf)
        mrow = const.tile([1, H], bf)
        hB = const.tile([H + 1, BL], bf)       # h+ bf16 state
        oB = const.tile([H + 1, BL], bf)       # o+ bf16 carry
        attnB = const.tile([H + 1, BL], bf)    # attn(t-1) bf16, row64 = 0
        sacc = const.tile([H, 2 * BL], f32)    # [s | acc]
        out_sb = const.tile([BL, t_steps * OUT], f32)

        for dst, src in ((xT, d_xT), (invm, d_invm), (m63, d_m63), (wh, d_wh),
                         (wx, d_wx), (mrow, d_mrow)):
            nc.sync.dma_start(out=dst, in_=src[:])

        nc.vector.memset(hB[0:H, :], 0.0)
        nc.vector.memset(hB[H:H + 1, :], 1.0)
        nc.vector.memset(oB[H:H + 1, :], 1.0)
        nc.vector.memset(attnB[H:H + 1, :], 0.0)
        nc.vector.memset(sacc, 0.0)

        h64 = hB[0:H, :]

        def gates(wcol_r, wcol_z, wcol_a, rhs2, rhs2_cols, mask_rhs, folded):
            """Gate psums + tanh front. rhs2: second rhs (xT slice / oB / attnB).
            Returns (tz, n) sbuf tiles (bf16)."""
            wc = wh if rhs2 is not xT else wx
            r2 = (rhs2[:, rhs2_cols] if rhs2 is xT else rhs2[:]) \
                if rhs2 is not None else None
            rz = p_rz.tile([H, 256], f32, tag="rz")
            rz_mms = [(rz[:, 0:128], wh[:, (_FR if folded else _HR):][:, 0:H], hB[:])]
            if r2 is not None:
                rz_mms.append((rz[:, 0:128], wc[:, wcol_r:wcol_r + H], r2))
            rz_mms.append((rz[:, 128:256], wh[:, (_FZ if folded else _HZ):][:, 0:H], hB[:]))
            if r2 is not None:
                rz_mms.append((rz[:, 128:256], wc[:, wcol_z:wcol_z + H], r2))
            if mask_rhs is not None:
                rz_mms.append((rz[:, 128:256], mrow[:], mask_rhs))
            for i, (o_, l_, r_) in enumerate(rz_mms):
                nc.tensor.matmul(o_, l_, r_, start=(i == 0),
                                 stop=(i == len(rz_mms) - 1))

            pa = p_a.tile([H, BL], f32, tag="pa")
            pa_mms = []
            if folded:
                pa_mms.append((wh[:, _FA2:_FA2 + H], hB[:]))
                if r2 is not None:
                    pa_mms.append((wc[:, wcol_a:wcol_a + H], r2))
            else:
                pa_mms.append((wc[:, wcol_a:wcol_a + H], r2))
            for i, (l_, r_) in enumerate(pa_mms):
                nc.tensor.matmul(pa[:], l_, r_, start=(i == 0),
                                 stop=(i == len(pa_mms) - 1))
            pb = p_b.tile([H, BL], f32, tag="pb")
            nc.tensor.matmul(pb[:], wh[:, _HB:_HB + H], hB[:],
                             start=True, stop=True)

            tr = temps.tile([H, BL], bf, tag="tr")
            nc.scalar.activation(out=tr, in_=rz[:, 0:128], func=AF.Tanh)
            tz = temps.tile([H, BL], bf, tag="tz")
            nc.scalar.activation(out=tz, in_=rz[:, 128:256], func=AF.Tanh)
            t2 = temps.tile([H, BL], f32, tag="t2")
            if folded:
                # t3 = tanh_r * B' + A2
                nc.vector.tensor_mul(out=t2, in0=tr, in1=pb[:])
            else:
                # t3 = (tanh_r + 1) * B' + A
                nc.vector.scalar_tensor_tensor(out=t2, in0=tr, scalar=1.0,
                                               in1=pb[:], op0=OP.add, op1=OP.mult)
            nc.vector.tensor_add(out=pb[:], in0=t2, in1=pa[:])
            n = temps.tile([H, BL], bf, tag="n")
            nc.scalar.activation(out=n, in_=pb[:], func=AF.Tanh)
            return tz, n

        def mix_pform(tz, n):
            """h' = 0.5(tz+1)n + 0.5h(1-tz), written to hB (bf16)."""
            p1 = temps.tile([H, BL], f32, tag="p1")
            nc.vector.scalar_tensor_tensor(out=p1, in0=tz, scalar=-0.5,
                                           in1=h64, op0=OP.mult, op1=OP.mult)
            pp = temps.tile([H, BL], f32, tag="pp")
            nc.vector.scalar_tensor_tensor(out=pp, in0=h64, scalar=0.5,
                                           in1=p1, op0=OP.mult, op1=OP.add)
            rr = temps.tile([H, BL], f32, tag="rr")
            nc.vector.scalar_tensor_tensor(out=rr, in0=tz, scalar=1.0,
                                           in1=n, op0=OP.add, op1=OP.mult)
            nc.vector.scalar_tensor_tensor(out=h64, in0=rr, scalar=0.5,
                                           in1=pp, op0=OP.mult, op1=OP.add)

        # ================= encoder =================
        for t in range(l_steps):
            mask_rhs = invm[:, t * BL:(t + 1) * BL] if t < l_steps - 1 else None
            tz, n = gates(_XR, _XZ, _XA, xT, slice(t * BL, (t + 1) * BL),
                          mask_rhs, folded=False)
            if t < l_steps - 1:
                mix_pform(tz, n)
            else:
                d = temps.tile([H, BL], f32, tag="d")
                nc.vector.tensor_sub(out=d, in0=n, in1=h64)
                tzd = temps.tile([H, BL], f32, tag="tzd")
                nc.vector.scalar_tensor_tensor(out=tzd, in0=tz, scalar=1.0,
                                               in1=d, op0=OP.add, op1=OP.mult)
                # out_last = m63 * (h + 0.5 tzd)  -> oB;  h_final = h + 0.5 m63 tzd
                hn = temps.tile([H, BL], f32, tag="hn")
                nc.vector.scalar_tensor_tensor(out=hn, in0=tzd, scalar=0.5,
                                               in1=h64, op0=OP.mult, op1=OP.add)
                u = temps.tile([H, BL], f32, tag="u")
                nc.vector.tensor_mul(out=u, in0=m63, in1=tzd)
                nc.vector.scalar_tensor_tensor(out=h64, in0=u, scalar=0.5,
                                               in1=h64, op0=OP.mult, op1=OP.add)
                nc.vector.tensor_mul(out=oB[0:H, :], in0=hn, in1=m63)

        # ================= decoder =================
        for t in range(t_steps):
            if t == 0:
                tz, n = gates(_CR, _CZ, _CA, oB, None, None, folded=False)
            elif t == 1:
                tz, n = gates(_CR, _CZ, _CA, None, None, None, folded=True)
            else:
                tz, n = gates(_CR, _CZ, _CA, attnB, None, None, folded=True)
            if t > 0:
                rec = temps.tile([H, BL], f32, tag="rec")
                nc.vector.reciprocal_approx_fast(out=rec, in_=sacc[:, 0:BL])
                attnt = temps.tile([H, BL], f32, tag="attnt")
                nc.vector.tensor_mul(out=attnt, in0=sacc[:, BL:2 * BL], in1=rec)
            mix_pform(tz, n)
            if t == 0:
                nc.vector.tensor_copy(out=oB[0:H, :], in_=h64)
            else:
                nc.vector.tensor_add(out=oB[0:H, :], in0=h64, in1=attnt)
                nc.vector.tensor_copy(out=attnB[0:H, :], in_=attnt)
            ly = p_ly.tile([BL, 141], f32, tag="ly")
            nc.tensor.matmul(ly[0:H, 0:128], wh[:, _WA:_WA + H], oB[:],
                             start=True, stop=True)
            nc.tensor.matmul(ly[0:BL, 128:141], oB[:], wh[:, _WF:_WF + OUT],
                             start=True, stop=True)
            eeo = temps.tile([H, 2 * BL], f32, tag="eeo")
            nc.scalar.activation(out=eeo[:, 0:BL], in_=ly[0:H, 0:128], func=AF.Exp)
            nc.gpsimd.tensor_mul(out=eeo[:, BL:2 * BL], in0=eeo[:, 0:BL],
                                 in1=oB[0:H, :])
            nc.gpsimd.tensor_add(out=sacc, in0=sacc[:], in1=eeo)
            nc.scalar.copy(out=out_sb[:, t * OUT:(t + 1) * OUT],
                           in_=ly[0:BL, 128:141])

        nc.sync.dma_start(out=d_out[:], in_=out_sb)
    if compile:
        nc.compile()
    return nc


def _make_in_maps(inputs, l_steps=L, t_steps=T):
    x = np.asarray(inputs["x"], np.float32)
    lengths = np.asarray(inputs["lengths"])
    w = _prep_weights(inputs["Wih"], inputs["Whh"], inputs["bih"],
                      inputs["bhh"], inputs["Wf"], inputs["bf"],
                      inputs["Wa"], inputs["ba"])
    in_maps = []
    for c in range(NCORES):
        sl = slice(c * BL, (c + 1) * BL)
        xT, invm, m63 = _prep_core(x[sl], lengths[sl], l_steps)
        in_maps.append(dict(xT=xT, invm=invm, m63=m63, **w))
    return in_maps


def kernel(**inputs):
    global LAST_EXEC_NS, TRACE_DIR
    from concourse.bass_utils import run_bass_kernel_spmd
    t_steps = int(inputs.get("output_length", T))
    assert t_steps == T, f"hardcoded for output_length={T}, got {t_steps}"
    nc = build_nc()
    in_maps = _make_in_maps(inputs)
    kw = {}
    if TRACE:
        import tempfile
        TRACE_DIR = tempfile.mkdtemp(prefix="bass_trace_")
        kw = dict(trace=True, tmpdir=TRACE_DIR)
    res = run_bass_kernel_spmd(nc, in_maps, list(range(NCORES)), **kw)
    LAST_EXEC_NS = res.exec_time_ns
    outs = [np.asarray(res.results[c]["out"]).reshape(BL, T, OUT)
            for c in range(NCORES)]
    return np.concatenate(outs, axis=0)


# revision 13
# speedup vs baseline: 1.5971x; 1.0229x over previous
"""Trainium2 Bass kernel for nn_AutoregressiveGRUWithAttention.

Strategy (data-parallel over batch, 8 cores x 128 batch):
  Feature-on-partition ("transposed") layout -> zero on-device transposes.
  States kept bf16-only: hB [65,128] (row64==1), oB [65,128], attnB [65,128]
  (row64==0). All matmuls bf16 (fp32 mm on TRN2 = 2 half-rate passes).
  Gates via tanh only (ACT set `exp_and_others`): sigmoid(v)=0.5+0.5tanh(v/2),
  0.5 folded into r/z weights, z negated so tanh gives zc=1-z.
  Decoder input o(t-1)=h(t-1)+attn(t-1) is never materialized for the gate
  matmuls: folded weights FR=HR+CR etc. act on h, and CR/CZ/CA act on the
  (early-available) attnB(t-1) -> only ONE matmul on the recurrence chain.
  n-gate: t3 = tanh_r*B' + A2 with A2=(CA+HB)-fold; t3 written back into B's
  psum bank by DVE; h-mix in P-form: h' = 0.5(tz+1)n + P, P=0.5h(1-tz)
  computed off-chain early.
  Encoder valid-length mask folded into z-gate via K=1 matmul of
  -0.5*BIG*(1-valid); last encoder step handled explicitly (d-form).
  Attention softmax streamed unnormalized (max|logit| ~ 1.4):
    sacc[64,256]=[s|acc]; eeo[64,256]=[exp(l)|exp(l)*o]; one fused GpSimd add;
    attn = acc * recip_approx_fast(s).
  y head: transposed-in-PSUM via swapped operands (lhsT=oB, rhs=WF bf16).

PSUM banks: rz[64,256] r|z, pa[64,128], pb[64,128] (B then t3), ly[128,141].
"""
import numpy as np
import ml_dtypes

B, L, T, IN, H, OUT = 1024, 64, 128, 13, 64, 13
NCORES, BL = 8, 128
BIG = 60.0
BF16 = ml_dtypes.bfloat16

# wh pack column offsets [65 x 653] (bf16)
_HR, _HZ, _HB, _CR, _CZ, _CA, _WA = 0, 64, 128, 192, 256, 320, 384
_FR, _FZ, _FA2, _WF = 448, 512, 576, 640
_WH_COLS = 653
# wx pack column offsets [14 x 192] (bf16)
_XR, _XZ, _XA = 0, 64, 128
_WX_COLS = 192

LAST_EXEC_NS = None
TRACE = False
TRACE_DIR = None
WARM_DUMMIES = 0


def _prep_weights(Wih, Whh, bih, bhh, Wf, bf, Wa, ba):
    f8 = np.float64
    Wih, Whh, bih, bhh, Wf, bf, Wa, ba = [np.asarray(a, f8) for a in
                                          (Wih, Whh, bih, bhh, Wf, bf, Wa, ba)]
    Wr, Wz, Wn = Wih[0:H], Wih[H:2 * H], Wih[2 * H:3 * H]
    Ur, Uz, Un = Whh[0:H], Whh[H:2 * H], Whh[2 * H:3 * H]
    br_i, bz_i, bn_i = bih[0:H], bih[H:2 * H], bih[2 * H:3 * H]
    br_h, bz_h, bn_h = bhh[0:H], bhh[H:2 * H], bhh[2 * H:3 * H]

    def blk(rows, rowbias, scale):
        m = np.zeros((H + 1, rows.shape[0]), f8)
        m[0:H] = scale * rows.T
        m[H] = scale * rowbias
        return m

    HRm = blk(Ur, br_i + br_h, 0.5)
    HZm = blk(Uz, bz_i + bz_h, -0.5)
    HBm = blk(Un, bn_h, 0.5)                        # B' = 0.5*(nh + bhh_n)
    CRm = blk((Wf.T @ Wr.T).T, bf @ Wr.T, 0.5)
    CZm = blk((Wf.T @ Wz.T).T, bf @ Wz.T, -0.5)
    CAm = blk((Wf.T @ Wn.T).T, bf @ Wn.T + bn_i, 1.0)
    WAm = blk(Wa, ba, 1.0)

    wh = np.zeros((H + 1, _WH_COLS), f8)
    for col, m in ((_HR, HRm), (_HZ, HZm), (_HB, HBm), (_CR, CRm), (_CZ, CZm),
                   (_CA, CAm), (_WA, WAm), (_FR, HRm + CRm), (_FZ, HZm + CZm),
                   (_FA2, CAm + HBm)):
        wh[:, col:col + H] = m
    wh[0:H, _WF:_WF + OUT] = Wf.T
    wh[H, _WF:_WF + OUT] = bf

    wx = np.zeros((IN + 1, _WX_COLS), f8)
    wx[0:IN, _XR:_XR + H] = 0.5 * Wr.T
    wx[0:IN, _XZ:_XZ + H] = -0.5 * Wz.T
    wx[0:IN, _XA:_XA + H] = Wn.T
    wx[IN, _XA:_XA + H] = bn_i

    mrow = np.full((1, H), -0.5 * BIG, f8)
    return dict(
        wh=np.ascontiguousarray(wh, BF16),
        wx=np.ascontiguousarray(wx, BF16),
        mrow=np.ascontiguousarray(mrow, BF16),
    )


def _prep_core(x_core, len_core, l_steps=L):
    x_core = np.asarray(x_core, np.float32)
    xT = np.zeros((IN + 1, l_steps, BL), np.float32)
    xT[0:IN] = np.transpose(x_core[:, 0:l_steps, :], (2, 1, 0))
    xT[IN] = 1.0
    valid = (np.arange(l_steps)[:, None] < np.asarray(len_core)[None, :])
    invm = (1.0 - valid.astype(np.float32)).reshape(1, l_steps * BL)
    m63 = valid[l_steps - 1].astype(np.float32)
    m63bc = np.ascontiguousarray(np.broadcast_to(m63, (H, BL)), np.float32)
    return (np.ascontiguousarray(xT.reshape(IN + 1, l_steps * BL), BF16),
            np.ascontiguousarray(invm, BF16), m63bc)


def build_nc(l_steps=L, t_steps=T, compile=True):
    import concourse.bacc as bacc
    import concourse.tile as tile
    from concourse import mybir
    from contextlib import ExitStack

    f32 = mybir.dt.float32
    bf = mybir.dt.bfloat16
    AF = mybir.ActivationFunctionType
    OP = mybir.AluOpType

    nc = bacc.Bacc("TRN2", target_bir_lowering=False, debug=False,
                   num_devices=NCORES)
    d_xT = nc.declare_dram_parameter("xT", [IN + 1, l_steps * BL], bf, isOutput=False)
    d_invm = nc.declare_dram_parameter("invm", [1, l_steps * BL], bf, isOutput=False)
    d_m63 = nc.declare_dram_parameter("m63", [H, BL], f32, isOutput=False)
    d_wh = nc.declare_dram_parameter("wh", [H + 1, _WH_COLS], bf, isOutput=False)
    d_wx = nc.declare_dram_parameter("wx", [IN + 1, _WX_COLS], bf, isOutput=False)
    d_mrow = nc.declare_dram_parameter("mrow", [1, H], bf, isOutput=False)
    d_out = nc.declare_dram_parameter("out", [BL, t_steps * OUT], f32, isOutput=True)

    with tile.TileContext(nc) as tc, ExitStack() as ctx:
        const = ctx.enter_context(tc.tile_pool(name="const", bufs=1))
        temps = ctx.enter_context(tc.tile_pool(name="temps", bufs=3))
        p_r = ctx.enter_context(tc.tile_pool(name="p_r", bufs=2, space="PSUM"))
        p_z = ctx.enter_context(tc.tile_pool(name="p_z", bufs=2, space="PSUM"))
        p_a = ctx.enter_context(tc.tile_pool(name="p_a", bufs=1, space="PSUM"))
        p_b = ctx.enter_context(tc.tile_pool(name="p_b", bufs=1, space="PSUM"))
        p_ly = ctx.enter_context(tc.tile_pool(name="p_ly", bufs=2, space="PSUM"))

        xT = const.tile([IN + 1, l_steps * BL], bf)
        invm = const.tile([1, l_steps * BL], bf)
        m63 = const.tile([H, BL], f32)
        wh = const.tile([H + 1, _WH_COLS], bf)
        wx = const.tile([IN + 1, _WX_COLS], bf)
        mrow = const.tile([1, H], bf)
        hB = const.tile([H + 1, BL], bf)       # h+ bf16 state
        oB = const.tile([H + 1, BL], bf)       # o+ bf16 carry
        attnB = const.tile([H + 1, BL], bf)    # attn(t-1) bf16, row64 = 0
        sacc = const.tile([H, 2 * BL], f32)    # [s | acc]
        out_sb = const.tile([BL, t_steps * OUT], f32)

        for dst, src in ((xT, d_xT), (invm, d_invm), (m63, d_m63), (wh, d_wh),
                         (wx, d_wx), (mrow, d_mrow)):
            nc.sync.dma_start(out=dst, in_=src[:])

        nc.vector.memset(hB[0:H, :], 0.0)
        nc.vector.memset(hB[H:H + 1, :], 1.0)
        nc.vector.memset(oB[H:H + 1, :], 1.0)
        nc.vector.memset(attnB[H:H + 1, :], 0.0)
        nc.vector.memset(sacc, 0.0)

        h64 = hB[0:H, :]

        def gate_mms(wcol_r, wcol_z, wcol_a, rhs2, rhs2_cols, mask_rhs, folded):
            """h-side gate matmuls first, then rhs2-side. Returns (pr, pz, pa, pb)."""
            wc = wh if rhs2 is not xT else wx
            r2 = (rhs2[:, rhs2_cols] if rhs2 is xT else rhs2[:]) \
                if rhs2 is not None else None
            pr = p_r.tile([H, BL], f32, tag="pr")
            pz = p_z.tile([H, BL], f32, tag="pz")
            pa = p_a.tile([H, BL], f32, tag="pa")
            pb = p_b.tile([H, BL], f32, tag="pb")
            one_r = r2 is None
            one_z = (r2 is None) and (mask_rhs is None)
            nc.tensor.matmul(pr[:], wh[:, (_FR if folded else _HR):][:, 0:H],
                             hB[:], start=True, stop=one_r)
            nc.tensor.matmul(pz[:], wh[:, (_FZ if folded else _HZ):][:, 0:H],
                             hB[:], start=True, stop=one_z)
            if folded:
                nc.tensor.matmul(pa[:], wh[:, _FA2:_FA2 + H], hB[:],
                                 start=True, stop=one_r)
            nc.tensor.matmul(pb[:], wh[:, _HB:_HB + H], hB[:],
                             start=True, stop=True)
            if r2 is not None:
                nc.tensor.matmul(pr[:], wc[:, wcol_r:wcol_r + H], r2,
                                 start=False, stop=True)
                nc.tensor.matmul(pz[:], wc[:, wcol_z:wcol_z + H], r2,
                                 start=False, stop=(mask_rhs is None))
                if folded:
                    nc.tensor.matmul(pa[:], wc[:, wcol_a:wcol_a + H], r2,
                                     start=False, stop=True)
                else:
                    nc.tensor.matmul(pa[:], wc[:, wcol_a:wcol_a + H], r2,
                                     start=True, stop=True)
            if mask_rhs is not None:
                nc.tensor.matmul(pz[:], mrow[:], mask_rhs, start=False, stop=True)
            return pr, pz, pa, pb

        def gate_front(pr, pz, pa, pb, folded):
            """tanh_r/z + t2 + t3(into pb). Returns (tz, t3psum)."""
            tr = temps.tile([H, BL], bf, tag="tr")
            nc.scalar.activation(out=tr, in_=pr[:], func=AF.Tanh)
            tz = temps.tile([H, BL], bf, tag="tz")
            nc.scalar.activation(out=tz, in_=pz[:], func=AF.Tanh)
            t2 = temps.tile([H, BL], f32, tag="t2")
            if folded:
                nc.vector.tensor_mul(out=t2, in0=tr, in1=pb[:])
            else:
                nc.vector.scalar_tensor_tensor(out=t2, in0=tr, scalar=1.0,
                                               in1=pb[:], op0=OP.add, op1=OP.mult)
            nc.vector.tensor_add(out=pb[:], in0=t2, in1=pa[:])
            return tz

        def mix_tail(tz, pb_t3):
            """p1, tanh_n, pp, rr, h' -> hB."""
            p1 = temps.tile([H, BL], f32, tag="p1")
            nc.vector.scalar_tensor_tensor(out=p1, in0=tz, scalar=-0.5,
                                           in1=h64, op0=OP.mult, op1=OP.mult)
            n = temps.tile([H, BL], bf, tag="n")
            nc.scalar.activation(out=n, in_=pb_t3[:], func=AF.Tanh)
            pp = temps.tile([H, BL], f32, tag="pp")
            nc.vector.scalar_tensor_tensor(out=pp, in0=h64, scalar=0.5,
                                           in1=p1, op0=OP.mult, op1=OP.add)
            rr = temps.tile([H, BL], f32, tag="rr")
            nc.vector.scalar_tensor_tensor(out=rr, in0=tz, scalar=1.0,
                                           in1=n, op0=OP.add, op1=OP.mult)
            nc.vector.scalar_tensor_tensor(out=h64, in0=rr, scalar=0.5,
                                           in1=pp, op0=OP.mult, op1=OP.add)
            return n

        def emit_tail(t):
            """Attention tail + output for decoder step t (deferred emission)."""
            ly = p_ly.tile([BL, 141], f32, tag="ly")
            nc.tensor.matmul(ly[0:H, 0:128], wh[:, _WA:_WA + H], oB[:],
                             start=True, stop=True)
            nc.tensor.matmul(ly[0:BL, 128:141], oB[:], wh[:, _WF:_WF + OUT],
                             start=True, stop=True)
            eeo = temps.tile([H, 2 * BL], f32, tag="eeo")
            nc.scalar.activation(out=eeo[:, 0:BL], in_=ly[0:H, 0:128], func=AF.Exp)
            nc.gpsimd.tensor_mul(out=eeo[:, BL:2 * BL], in0=eeo[:, 0:BL],
                                 in1=oB[0:H, :])
            nc.gpsimd.tensor_add(out=sacc, in0=sacc[:], in1=eeo)
            nc.scalar.copy(out=out_sb[:, t * OUT:(t + 1) * OUT],
                           in_=ly[0:BL, 128:141])

        # ================= encoder =================
        for t in range(l_steps):
            mask_rhs = invm[:, t * BL:(t + 1) * BL] if t < l_steps - 1 else None
            pr, pz, pa, pb = gate_mms(_XR, _XZ, _XA, xT,
                                      slice(t * BL, (t + 1) * BL), mask_rhs, False)
            tz = gate_front(pr, pz, pa, pb, False)
            if t < l_steps - 1:
                mix_tail(tz, pb)
            else:
                n = temps.tile([H, BL], bf, tag="n")
                nc.scalar.activation(out=n, in_=pb[:], func=AF.Tanh)
                d = temps.tile([H, BL], f32, tag="d")
                nc.vector.tensor_sub(out=d, in0=n, in1=h64)
                tzd = temps.tile([H, BL], f32, tag="tzd")
                nc.vector.scalar_tensor_tensor(out=tzd, in0=tz, scalar=1.0,
                                               in1=d, op0=OP.add, op1=OP.mult)
                hn = temps.tile([H, BL], f32, tag="hn")
                nc.vector.scalar_tensor_tensor(out=hn, in0=tzd, scalar=0.5,
                                               in1=h64, op0=OP.mult, op1=OP.add)
                u = temps.tile([H, BL], f32, tag="u")
                nc.vector.tensor_mul(out=u, in0=m63, in1=tzd)
                nc.vector.scalar_tensor_tensor(out=h64, in0=u, scalar=0.5,
                                               in1=h64, op0=OP.mult, op1=OP.add)
                nc.vector.tensor_mul(out=oB[0:H, :], in0=hn, in1=m63)

        # ================= decoder (tail of step t-1 emitted inside step t) ===
        for t in range(t_steps):
            if t == 0:
                pr, pz, pa, pb = gate_mms(_CR, _CZ, _CA, oB, None, None, False)
            elif t == 1:
                pr, pz, pa, pb = gate_mms(_CR, _CZ, _CA, None, None, None, True)
            else:
                pr, pz, pa, pb = gate_mms(_CR, _CZ, _CA, attnB, None, None, True)
            tz = gate_front(pr, pz, pa, pb, folded=(t > 0))
            if t > 0:
                emit_tail(t - 1)          # ly/exp/eo/sacc/ycopy of prev step
            mix_tail(tz, pb)
            if t == 0:
                nc.vector.tensor_copy(out=oB[0:H, :], in_=h64)
            else:
                rec = temps.tile([H, BL], f32, tag="rec")
                nc.vector.reciprocal_approx_fast(out=rec, in_=sacc[:, 0:BL])
                nc.vector.tensor_mul(out=attnB[0:H, :], in0=sacc[:, BL:2 * BL],
                                     in1=rec)
                nc.gpsimd.tensor_add(out=oB[0:H, :], in0=h64, in1=attnB[0:H, :])
        emit_tail(t_steps - 1)

        nc.sync.dma_start(out=d_out[:], in_=out_sb)
    if compile:
        nc.compile()
    return nc


def _make_in_maps(inputs, l_steps=L, t_steps=T):
    x = np.asarray(inputs["x"], np.float32)
    lengths = np.asarray(inputs["lengths"])
    w = _prep_weights(inputs["Wih"], inputs["Whh"], inputs["bih"],
                      inputs["bhh"], inputs["Wf"], inputs["bf"],
                      inputs["Wa"], inputs["ba"])
    in_maps = []
    for c in range(NCORES):
        sl = slice(c * BL, (c + 1) * BL)
        xT, invm, m63 = _prep_core(x[sl], lengths[sl], l_steps)
        in_maps.append(dict(xT=xT, invm=invm, m63=m63, **w))
    return in_maps


def kernel(**inputs):
    global LAST_EXEC_NS, TRACE_DIR
    from concourse.bass_utils import run_bass_kernel_spmd
    t_steps = int(inputs.get("output_length", T))
    assert t_steps == T, f"hardcoded for output_length={T}, got {t_steps}"
    nc = build_nc()
    in_maps = _make_in_maps(inputs)
    kw = {}
    if TRACE:
        import tempfile
        TRACE_DIR = tempfile.mkdtemp(prefix="bass_trace_")
        kw = dict(trace=True, tmpdir=TRACE_DIR)
    res = run_bass_kernel_spmd(nc, in_maps, list(range(NCORES)), **kw)
    LAST_EXEC_NS = res.exec_time_ns
    outs = [np.asarray(res.results[c]["out"]).reshape(BL, T, OUT)
            for c in range(NCORES)]
    return np.concatenate(outs, axis=0)


# revision 17
# speedup vs baseline: 1.8081x; 1.1321x over previous
"""Trainium2 Bass kernel for nn_AutoregressiveGRUWithAttention.

Strategy (data-parallel over batch, 8 cores x 128 batch):
  Feature-on-partition ("transposed") layout -> zero on-device transposes.
  States kept bf16-only: hB [65,128] (row64==1), oB [65,128], attnB [65,128]
  (row64==0). All matmuls bf16 (fp32 mm on TRN2 = 2 half-rate passes).
  Gates via tanh only (ACT set `exp_and_others`): sigmoid(v)=0.5+0.5tanh(v/2),
  0.5 folded into r/z weights, z negated so tanh gives zc=1-z.
  Decoder input o(t-1)=h(t-1)+attn(t-1) is never materialized for the gate
  matmuls: folded weights FR=HR+CR etc. act on h, and CR/CZ/CA act on the
  (early-available) attnB(t-1) -> only ONE matmul on the recurrence chain.
  n-gate: t3 = tanh_r*B' + A2 with A2=(CA+HB)-fold; t3 written back into B's
  psum bank by DVE; h-mix in P-form: h' = 0.5(tz+1)n + P, P=0.5h(1-tz)
  computed off-chain early.
  Encoder valid-length mask folded into z-gate via K=1 matmul of
  -0.5*BIG*(1-valid); last encoder step handled explicitly (d-form).
  Attention softmax streamed unnormalized (max|logit| ~ 1.4):
    sacc[64,256]=[s|acc]; eeo[64,256]=[exp(l)|exp(l)*o]; one fused GpSimd add;
    attn = acc * recip_approx_fast(s).
  y head: transposed-in-PSUM via swapped operands (lhsT=oB, rhs=WF bf16).

PSUM banks: rz[64,256] r|z, pa[64,128], pb[64,128] (B then t3), ly[128,141].
"""
import numpy as np
import ml_dtypes

B, L, T, IN, H, OUT = 1024, 64, 128, 13, 64, 13
NCORES, BL = 8, 128
BIG = 60.0
BF16 = ml_dtypes.bfloat16

# wh pack column offsets [65 x 653] (bf16)
_HR, _HZ, _HB, _CR, _CZ, _CA, _WA = 0, 64, 128, 192, 256, 320, 384
_FR, _FZ, _FA2, _WF = 448, 512, 576, 640
_WH_COLS = 653
# wx pack column offsets [14 x 192] (bf16)
_XR, _XZ, _XA = 0, 64, 128
_WX_COLS = 192

LAST_EXEC_NS = None
TRACE = False
TRACE_DIR = None
WARM_DUMMIES = 0


def _prep_weights(Wih, Whh, bih, bhh, Wf, bf, Wa, ba):
    f8 = np.float64
    Wih, Whh, bih, bhh, Wf, bf, Wa, ba = [np.asarray(a, f8) for a in
                                          (Wih, Whh, bih, bhh, Wf, bf, Wa, ba)]
    Wr, Wz, Wn = Wih[0:H], Wih[H:2 * H], Wih[2 * H:3 * H]
    Ur, Uz, Un = Whh[0:H], Whh[H:2 * H], Whh[2 * H:3 * H]
    br_i, bz_i, bn_i = bih[0:H], bih[H:2 * H], bih[2 * H:3 * H]
    br_h, bz_h, bn_h = bhh[0:H], bhh[H:2 * H], bhh[2 * H:3 * H]

    def blk(rows, rowbias, scale):
        m = np.zeros((H + 1, rows.shape[0]), f8)
        m[0:H] = scale * rows.T
        m[H] = scale * rowbias
        return m

    HRm = blk(Ur, br_i + br_h, 0.5)
    HZm = blk(Uz, bz_i + bz_h, -0.5)
    HBm = blk(Un, bn_h, 0.5)                        # B' = 0.5*(nh + bhh_n)
    CRm = blk((Wf.T @ Wr.T).T, bf @ Wr.T, 0.5)
    CZm = blk((Wf.T @ Wz.T).T, bf @ Wz.T, -0.5)
    CAm = blk((Wf.T @ Wn.T).T, bf @ Wn.T + bn_i, 1.0)
    WAm = blk(Wa, ba, 1.0)

    wh = np.zeros((H + 1, _WH_COLS), f8)
    for col, m in ((_HR, HRm), (_HZ, HZm), (_HB, HBm), (_CR, CRm), (_CZ, CZm),
                   (_CA, CAm), (_WA, WAm), (_FR, HRm + CRm), (_FZ, HZm + CZm),
                   (_FA2, CAm + HBm)):
        wh[:, col:col + H] = m
    wh[0:H, _WF:_WF + OUT] = Wf.T
    wh[H, _WF:_WF + OUT] = bf

    wx = np.zeros((IN + 1, _WX_COLS), f8)
    wx[0:IN, _XR:_XR + H] = 0.5 * Wr.T
    wx[0:IN, _XZ:_XZ + H] = -0.5 * Wz.T
    wx[0:IN, _XA:_XA + H] = Wn.T
    wx[IN, _XA:_XA + H] = bn_i

    mrow = np.full((1, H), -0.5 * BIG, f8)
    return dict(
        wh=np.ascontiguousarray(wh, BF16),
        wx=np.ascontiguousarray(wx, BF16),
        mrow=np.ascontiguousarray(mrow, BF16),
    )


def _prep_core(x_core, len_core, l_steps=L):
    x_core = np.asarray(x_core, np.float32)
    xT = np.zeros((IN + 1, l_steps, BL), np.float32)
    xT[0:IN] = np.transpose(x_core[:, 0:l_steps, :], (2, 1, 0))
    xT[IN] = 1.0
    valid = (np.arange(l_steps)[:, None] < np.asarray(len_core)[None, :])
    invm = (1.0 - valid.astype(np.float32)).reshape(1, l_steps * BL)
    m63 = valid[l_steps - 1].astype(np.float32)
    m63bc = np.ascontiguousarray(np.broadcast_to(m63, (H, BL)), np.float32)
    return (np.ascontiguousarray(xT.reshape(IN + 1, l_steps * BL), BF16),
            np.ascontiguousarray(invm, BF16), m63bc)


def build_nc(l_steps=L, t_steps=T, compile=True):
    import concourse.bacc as bacc
    import concourse.tile as tile
    from concourse import mybir
    from contextlib import ExitStack

    f32 = mybir.dt.float32
    bf = mybir.dt.bfloat16
    AF = mybir.ActivationFunctionType
    OP = mybir.AluOpType

    nc = bacc.Bacc("TRN2", target_bir_lowering=False, debug=False,
                   num_devices=NCORES)
    d_xT = nc.declare_dram_parameter("xT", [IN + 1, l_steps * BL], bf, isOutput=False)
    d_invm = nc.declare_dram_parameter("invm", [1, l_steps * BL], bf, isOutput=False)
    d_m63 = nc.declare_dram_parameter("m63", [H, BL], f32, isOutput=False)
    d_wh = nc.declare_dram_parameter("wh", [H + 1, _WH_COLS], bf, isOutput=False)
    d_wx = nc.declare_dram_parameter("wx", [IN + 1, _WX_COLS], bf, isOutput=False)
    d_mrow = nc.declare_dram_parameter("mrow", [1, H], bf, isOutput=False)
    d_out = nc.declare_dram_parameter("out", [BL, t_steps * OUT], f32, isOutput=True)

    with tile.TileContext(nc) as tc, ExitStack() as ctx:
        const = ctx.enter_context(tc.tile_pool(name="const", bufs=1))
        temps = ctx.enter_context(tc.tile_pool(name="temps", bufs=3))
        p_r = ctx.enter_context(tc.tile_pool(name="p_r", bufs=2, space="PSUM"))
        p_z = ctx.enter_context(tc.tile_pool(name="p_z", bufs=2, space="PSUM"))
        p_a = ctx.enter_context(tc.tile_pool(name="p_a", bufs=1, space="PSUM"))
        p_b = ctx.enter_context(tc.tile_pool(name="p_b", bufs=1, space="PSUM"))
        p_ly = ctx.enter_context(tc.tile_pool(name="p_ly", bufs=2, space="PSUM"))

        xT = const.tile([IN + 1, l_steps * BL], bf)
        invm = const.tile([1, l_steps * BL], bf)
        m63 = const.tile([H, BL], f32)
        wh = const.tile([H + 1, _WH_COLS], bf)
        wx = const.tile([IN + 1, _WX_COLS], bf)
        mrow = const.tile([1, H], bf)
        hB = const.tile([H + 1, BL], bf)       # h+ bf16 state
        oB = const.tile([H + 1, BL], bf)       # o+ bf16 carry
        attnB = const.tile([H + 1, BL], bf)    # attn(t-1) bf16, row64 = 0
        sacc = const.tile([H, 2 * BL], f32)    # [s | acc]
        out_sb = const.tile([BL, t_steps * OUT], f32)

        for dst, src in ((xT, d_xT), (invm, d_invm), (m63, d_m63), (wh, d_wh),
                         (wx, d_wx), (mrow, d_mrow)):
            nc.sync.dma_start(out=dst, in_=src[:])

        nc.vector.memset(hB[0:H, :], 0.0)
        nc.vector.memset(hB[H:H + 1, :], 1.0)
        nc.vector.memset(oB[H:H + 1, :], 1.0)
        nc.vector.memset(attnB[H:H + 1, :], 0.0)
        nc.vector.memset(sacc, 0.0)

        h64 = hB[0:H, :]

        def gate_mms(wcol_r, wcol_z, wcol_a, rhs2, rhs2_cols, mask_rhs, folded):
            """h-side gate matmuls first, then rhs2-side. Returns (pr, pz, pa, pb)."""
            wc = wh if rhs2 is not xT else wx
            r2 = (rhs2[:, rhs2_cols] if rhs2 is xT else rhs2[:]) \
                if rhs2 is not None else None
            pr = p_r.tile([H, BL], f32, tag="pr")
            pz = p_z.tile([H, BL], f32, tag="pz")
            pa = p_a.tile([H, BL], f32, tag="pa")
            pb = p_b.tile([H, BL], f32, tag="pb")
            one_r = r2 is None
            one_z = (r2 is None) and (mask_rhs is None)
            # r-group first and closed ASAP (tanh_r is on the critical chain)
            nc.tensor.matmul(pr[:], wh[:, (_FR if folded else _HR):][:, 0:H],
                             hB[:], start=True, stop=one_r)
            if r2 is not None:
                nc.tensor.matmul(pr[:], wc[:, wcol_r:wcol_r + H], r2,
                                 start=False, stop=True)
            nc.tensor.matmul(pz[:], wh[:, (_FZ if folded else _HZ):][:, 0:H],
                             hB[:], start=True, stop=one_z)
            if r2 is not None:
                nc.tensor.matmul(pz[:], wc[:, wcol_z:wcol_z + H], r2,
                                 start=False, stop=(mask_rhs is None))
            if mask_rhs is not None:
                nc.tensor.matmul(pz[:], mrow[:], mask_rhs, start=False, stop=True)
            nc.tensor.matmul(pb[:], wh[:, _HB:_HB + H], hB[:],
                             start=True, stop=True)
            if folded:
                nc.tensor.matmul(pa[:], wh[:, _FA2:_FA2 + H], hB[:],
                                 start=True, stop=one_r)
                if r2 is not None:
                    nc.tensor.matmul(pa[:], wc[:, wcol_a:wcol_a + H], r2,
                                     start=False, stop=True)
            else:
                nc.tensor.matmul(pa[:], wc[:, wcol_a:wcol_a + H], r2,
                                 start=True, stop=True)
            return pr, pz, pa, pb

        def gate_front(pr, pz, pa, pb, folded):
            """tanh_r/z + t2 + t3(into pb). Returns (tz, t3psum)."""
            tr = temps.tile([H, BL], bf, tag="tr")
            nc.scalar.activation(out=tr, in_=pr[:], func=AF.Tanh)
            tz = temps.tile([H, BL], bf, tag="tz")
            nc.scalar.activation(out=tz, in_=pz[:], func=AF.Tanh)
            t2 = temps.tile([H, BL], f32, tag="t2")
            if folded:
                nc.vector.tensor_mul(out=t2, in0=tr, in1=pb[:])
            else:
                nc.vector.scalar_tensor_tensor(out=t2, in0=tr, scalar=1.0,
                                               in1=pb[:], op0=OP.add, op1=OP.mult)
            nc.vector.tensor_add(out=pb[:], in0=t2, in1=pa[:])
            return tz

        def mix_tail(tz, pb_t3):
            """p1, tanh_n, pp, rr, h' -> hB."""
            p1 = temps.tile([H, BL], f32, tag="p1")
            nc.vector.scalar_tensor_tensor(out=p1, in0=tz, scalar=-0.5,
                                           in1=h64, op0=OP.mult, op1=OP.mult)
            n = temps.tile([H, BL], bf, tag="n")
            nc.scalar.activation(out=n, in_=pb_t3[:], func=AF.Tanh)
            pp = temps.tile([H, BL], f32, tag="pp")
            nc.vector.scalar_tensor_tensor(out=pp, in0=h64, scalar=0.5,
                                           in1=p1, op0=OP.mult, op1=OP.add)
            rr = temps.tile([H, BL], f32, tag="rr")
            nc.vector.scalar_tensor_tensor(out=rr, in0=tz, scalar=1.0,
                                           in1=n, op0=OP.add, op1=OP.mult)
            nc.vector.scalar_tensor_tensor(out=h64, in0=rr, scalar=0.5,
                                           in1=pp, op0=OP.mult, op1=OP.add)
            return n

        def emit_tail(t):
            """Attention tail + output for decoder step t (deferred emission)."""
            ly = p_ly.tile([BL, 141], f32, tag="ly")
            nc.tensor.matmul(ly[0:H, 0:128], wh[:, _WA:_WA + H], oB[:],
                             start=True, stop=True)
            nc.tensor.matmul(ly[0:BL, 128:141], oB[:], wh[:, _WF:_WF + OUT],
                             start=True, stop=True)
            e = temps.tile([H, BL], f32, tag="e")
            nc.scalar.activation(out=e, in_=ly[0:H, 0:128], func=AF.Exp)
            eo = temps.tile([H, BL], f32, tag="eo")
            nc.gpsimd.tensor_mul(out=eo, in0=e, in1=oB[0:H, :])
            nc.vector.tensor_add(out=sacc[:, 0:BL], in0=sacc[:, 0:BL], in1=e)
            nc.gpsimd.tensor_add(out=sacc[:, BL:2 * BL], in0=sacc[:, BL:2 * BL],
                                 in1=eo)
            return ly

        # ================= encoder =================
        for t in range(l_steps):
            mask_rhs = invm[:, t * BL:(t + 1) * BL] if t < l_steps - 1 else None
            pr, pz, pa, pb = gate_mms(_XR, _XZ, _XA, xT,
                                      slice(t * BL, (t + 1) * BL), mask_rhs, False)
            tz = gate_front(pr, pz, pa, pb, False)
            if t < l_steps - 1:
                mix_tail(tz, pb)
            else:
                n = temps.tile([H, BL], bf, tag="n")
                nc.scalar.activation(out=n, in_=pb[:], func=AF.Tanh)
                d = temps.tile([H, BL], f32, tag="d")
                nc.vector.tensor_sub(out=d, in0=n, in1=h64)
                tzd = temps.tile([H, BL], f32, tag="tzd")
                nc.vector.scalar_tensor_tensor(out=tzd, in0=tz, scalar=1.0,
                                               in1=d, op0=OP.add, op1=OP.mult)
                hn = temps.tile([H, BL], f32, tag="hn")
                nc.vector.scalar_tensor_tensor(out=hn, in0=tzd, scalar=0.5,
                                               in1=h64, op0=OP.mult, op1=OP.add)
                u = temps.tile([H, BL], f32, tag="u")
                nc.vector.tensor_mul(out=u, in0=m63, in1=tzd)
                nc.vector.scalar_tensor_tensor(out=h64, in0=u, scalar=0.5,
                                               in1=h64, op0=OP.mult, op1=OP.add)
                nc.vector.tensor_mul(out=oB[0:H, :], in0=hn, in1=m63)

        # ================= decoder (tail of step t-1 emitted inside step t) ===
        for t in range(t_steps):
            if t == 0:
                pr, pz, pa, pb = gate_mms(_CR, _CZ, _CA, oB, None, None, False)
            elif t == 1:
                pr, pz, pa, pb = gate_mms(_CR, _CZ, _CA, None, None, None, True)
            else:
                pr, pz, pa, pb = gate_mms(_CR, _CZ, _CA, attnB, None, None, True)
            tz = gate_front(pr, pz, pa, pb, folded=(t > 0))
            ly_prev = emit_tail(t - 1) if t > 0 else None
            mix_tail(tz, pb)
            if t == 0:
                nc.vector.tensor_copy(out=oB[0:H, :], in_=h64)
            else:
                rec = temps.tile([H, BL], f32, tag="rec")
                nc.vector.reciprocal_approx_fast(out=rec, in_=sacc[:, 0:BL])
                nc.vector.tensor_mul(out=attnB[0:H, :], in0=sacc[:, BL:2 * BL],
                                     in1=rec)
                nc.gpsimd.tensor_add(out=oB[0:H, :], in0=h64, in1=attnB[0:H, :])
            if ly_prev is not None:
                nc.scalar.copy(out=out_sb[:, (t - 1) * OUT:t * OUT],
                               in_=ly_prev[0:BL, 128:141])
        ly_last = emit_tail(t_steps - 1)
        nc.scalar.copy(out=out_sb[:, (t_steps - 1) * OUT:t_steps * OUT],
                       in_=ly_last[0:BL, 128:141])

        nc.sync.dma_start(out=d_out[:], in_=out_sb)
    if compile:
        nc.compile()
    return nc


def _make_in_maps(inputs, l_steps=L, t_steps=T):
    x = np.asarray(inputs["x"], np.float32)
    lengths = np.asarray(inputs["lengths"])
    w = _prep_weights(inputs["Wih"], inputs["Whh"], inputs["bih"],
                      inputs["bhh"], inputs["Wf"], inputs["bf"],
                      inputs["Wa"], inputs["ba"])
    in_maps = []
    for c in range(NCORES):
        sl = slice(c * BL, (c + 1) * BL)
        xT, invm, m63 = _prep_core(x[sl], lengths[sl], l_steps)
        in_maps.append(dict(xT=xT, invm=invm, m63=m63, **w))
    return in_maps


def kernel(**inputs):
    global LAST_EXEC_NS, TRACE_DIR
    from concourse.bass_utils import run_bass_kernel_spmd
    t_steps = int(inputs.get("output_length", T))
    assert t_steps == T, f"hardcoded for output_length={T}, got {t_steps}"
    nc = build_nc()
    in_maps = _make_in_maps(inputs)
    kw = {}
    if TRACE:
        import tempfile
        TRACE_DIR = tempfile.mkdtemp(prefix="bass_trace_")
        kw = dict(trace=True, tmpdir=TRACE_DIR)
    res = run_bass_kernel_spmd(nc, in_maps, list(range(NCORES)), **kw)
    LAST_EXEC_NS = res.exec_time_ns
    outs = [np.asarray(res.results[c]["out"]).reshape(BL, T, OUT)
            for c in range(NCORES)]
    return np.concatenate(outs, axis=0)


# revision 18
# speedup vs baseline: 1.9299x; 1.0674x over previous
"""Trainium2 Bass kernel for nn_AutoregressiveGRUWithAttention.

Strategy (data-parallel over batch, 8 cores x 128 batch):
  Feature-on-partition ("transposed") layout -> zero on-device transposes.
  States kept bf16-only: hB [65,128] (row64==1), oB [65,128], attnB [65,128]
  (row64==0). All matmuls bf16 (fp32 mm on TRN2 = 2 half-rate passes).
  Gates via tanh only (ACT set `exp_and_others`): sigmoid(v)=0.5+0.5tanh(v/2),
  0.5 folded into r/z weights, z negated so tanh gives zc=1-z.
  Decoder input o(t-1)=h(t-1)+attn(t-1) is never materialized for the gate
  matmuls: folded weights FR=HR+CR etc. act on h, and CR/CZ/CA act on the
  (early-available) attnB(t-1) -> only ONE matmul on the recurrence chain.
  n-gate: t3 = tanh_r*B' + A2 with A2=(CA+HB)-fold; t3 written back into B's
  psum bank by DVE; h-mix in P-form: h' = 0.5(tz+1)n + P, P=0.5h(1-tz)
  computed off-chain early.
  Encoder valid-length mask folded into z-gate via K=1 matmul of
  -0.5*BIG*(1-valid); last encoder step handled explicitly (d-form).
  Attention softmax streamed unnormalized (max|logit| ~ 1.4):
    sacc[64,256]=[s|acc]; eeo[64,256]=[exp(l)|exp(l)*o]; one fused GpSimd add;
    attn = acc * recip_approx_fast(s).
  y head: transposed-in-PSUM via swapped operands (lhsT=oB, rhs=WF bf16).

PSUM banks: rz[64,256] r|z, pa[64,128], pb[64,128] (B then t3), ly[128,141].
"""
import numpy as np
import ml_dtypes

B, L, T, IN, H, OUT = 1024, 64, 128, 13, 64, 13
NCORES, BL = 8, 128
BIG = 60.0
BF16 = ml_dtypes.bfloat16

# wh pack column offsets [65 x 653] (bf16)
_HR, _HZ, _HB, _CR, _CZ, _CA, _WA = 0, 64, 128, 192, 256, 320, 384
_FR, _FZ, _FA2, _WF = 448, 512, 576, 640
_WH_COLS = 653
# wx pack column offsets [14 x 192] (bf16)
_XR, _XZ, _XA = 0, 64, 128
_WX_COLS = 192

LAST_EXEC_NS = None
TRACE = False
TRACE_DIR = None
WARM_DUMMIES = 0


def _prep_weights(Wih, Whh, bih, bhh, Wf, bf, Wa, ba):
    f8 = np.float64
    Wih, Whh, bih, bhh, Wf, bf, Wa, ba = [np.asarray(a, f8) for a in
                                          (Wih, Whh, bih, bhh, Wf, bf, Wa, ba)]
    Wr, Wz, Wn = Wih[0:H], Wih[H:2 * H], Wih[2 * H:3 * H]
    Ur, Uz, Un = Whh[0:H], Whh[H:2 * H], Whh[2 * H:3 * H]
    br_i, bz_i, bn_i = bih[0:H], bih[H:2 * H], bih[2 * H:3 * H]
    br_h, bz_h, bn_h = bhh[0:H], bhh[H:2 * H], bhh[2 * H:3 * H]

    def blk(rows, rowbias, scale):
        m = np.zeros((H + 1, rows.shape[0]), f8)
        m[0:H] = scale * rows.T
        m[H] = scale * rowbias
        return m

    HRm = blk(Ur, br_i + br_h, 0.5)
    HZm = blk(Uz, bz_i + bz_h, -0.5)
    HBm = blk(Un, bn_h, 0.5)                        # B' = 0.5*(nh + bhh_n)
    CRm = blk((Wf.T @ Wr.T).T, bf @ Wr.T, 0.5)
    CZm = blk((Wf.T @ Wz.T).T, bf @ Wz.T, -0.5)
    CAm = blk((Wf.T @ Wn.T).T, bf @ Wn.T + bn_i, 1.0)
    WAm = blk(Wa, ba, 1.0)

    wh = np.zeros((H + 1, _WH_COLS), f8)
    for col, m in ((_HR, HRm), (_HZ, HZm), (_HB, HBm), (_CR, CRm), (_CZ, CZm),
                   (_CA, CAm), (_WA, WAm), (_FR, HRm + CRm), (_FZ, HZm + CZm),
                   (_FA2, CAm + HBm)):
        wh[:, col:col + H] = m
    wh[0:H, _WF:_WF + OUT] = Wf.T
    wh[H, _WF:_WF + OUT] = bf

    wx = np.zeros((IN + 1, _WX_COLS), f8)
    wx[0:IN, _XR:_XR + H] = 0.5 * Wr.T
    wx[0:IN, _XZ:_XZ + H] = -0.5 * Wz.T
    wx[0:IN, _XA:_XA + H] = Wn.T
    wx[IN, _XA:_XA + H] = bn_i

    mrow = np.full((1, H), -0.5 * BIG, f8)
    return dict(
        wh=np.ascontiguousarray(wh, BF16),
        wx=np.ascontiguousarray(wx, BF16),
        mrow=np.ascontiguousarray(mrow, BF16),
    )


def _prep_core(x_core, len_core, l_steps=L):
    x_core = np.asarray(x_core, np.float32)
    xT = np.zeros((IN + 1, l_steps, BL), np.float32)
    xT[0:IN] = np.transpose(x_core[:, 0:l_steps, :], (2, 1, 0))
    xT[IN] = 1.0
    valid = (np.arange(l_steps)[:, None] < np.asarray(len_core)[None, :])
    invm = (1.0 - valid.astype(np.float32)).reshape(1, l_steps * BL)
    m63 = valid[l_steps - 1].astype(np.float32)
    m63bc = np.ascontiguousarray(np.broadcast_to(m63, (H, BL)), np.float32)
    return (np.ascontiguousarray(xT.reshape(IN + 1, l_steps * BL), BF16),
            np.ascontiguousarray(invm, BF16), m63bc)


def build_nc(l_steps=L, t_steps=T, compile=True):
    import concourse.bacc as bacc
    import concourse.tile as tile
    from concourse import mybir
    from contextlib import ExitStack

    f32 = mybir.dt.float32
    bf = mybir.dt.bfloat16
    AF = mybir.ActivationFunctionType
    OP = mybir.AluOpType

    nc = bacc.Bacc("TRN2", target_bir_lowering=False, debug=False,
                   num_devices=NCORES)
    d_xT = nc.declare_dram_parameter("xT", [IN + 1, l_steps * BL], bf, isOutput=False)
    d_invm = nc.declare_dram_parameter("invm", [1, l_steps * BL], bf, isOutput=False)
    d_m63 = nc.declare_dram_parameter("m63", [H, BL], f32, isOutput=False)
    d_wh = nc.declare_dram_parameter("wh", [H + 1, _WH_COLS], bf, isOutput=False)
    d_wx = nc.declare_dram_parameter("wx", [IN + 1, _WX_COLS], bf, isOutput=False)
    d_mrow = nc.declare_dram_parameter("mrow", [1, H], bf, isOutput=False)
    d_out = nc.declare_dram_parameter("out", [BL, t_steps * OUT], f32, isOutput=True)

    with tile.TileContext(nc) as tc, ExitStack() as ctx:
        const = ctx.enter_context(tc.tile_pool(name="const", bufs=1))
        temps = ctx.enter_context(tc.tile_pool(name="temps", bufs=3))
        p_r = ctx.enter_context(tc.tile_pool(name="p_r", bufs=2, space="PSUM"))
        p_z = ctx.enter_context(tc.tile_pool(name="p_z", bufs=2, space="PSUM"))
        p_a = ctx.enter_context(tc.tile_pool(name="p_a", bufs=1, space="PSUM"))
        p_b = ctx.enter_context(tc.tile_pool(name="p_b", bufs=1, space="PSUM"))
        p_ly = ctx.enter_context(tc.tile_pool(name="p_ly", bufs=2, space="PSUM"))

        xT = const.tile([IN + 1, l_steps * BL], bf)
        invm = const.tile([1, l_steps * BL], bf)
        m63 = const.tile([H, BL], f32)
        wh = const.tile([H + 1, _WH_COLS], bf)
        wx = const.tile([IN + 1, _WX_COLS], bf)
        mrow = const.tile([1, H], bf)
        hB = const.tile([H + 1, BL], bf)       # h+ bf16 state
        oB = const.tile([H + 1, BL], bf)       # o+ bf16 carry
        attnB = const.tile([H + 1, BL], bf)    # attn(t-1) bf16, row64 = 0
        sacc = const.tile([H, 2 * BL], f32)    # [s | acc]
        out_sb = const.tile([BL, t_steps * OUT], f32)

        for dst, src in ((xT, d_xT), (invm, d_invm), (m63, d_m63), (wh, d_wh),
                         (wx, d_wx), (mrow, d_mrow)):
            nc.sync.dma_start(out=dst, in_=src[:])

        nc.vector.memset(hB[0:H, :], 0.0)
        nc.vector.memset(hB[H:H + 1, :], 1.0)
        nc.vector.memset(oB[H:H + 1, :], 1.0)
        nc.vector.memset(attnB[H:H + 1, :], 0.0)
        nc.vector.memset(sacc, 0.0)

        h64 = hB[0:H, :]

        def gate_mms(wcol_r, wcol_z, wcol_a, rhs2, rhs2_cols, mask_rhs, folded):
            """h-side gate matmuls first, then rhs2-side. Returns (pr, pz, pa, pb)."""
            wc = wh if rhs2 is not xT else wx
            r2 = (rhs2[:, rhs2_cols] if rhs2 is xT else rhs2[:]) \
                if rhs2 is not None else None
            pr = p_r.tile([H, BL], f32, tag="pr")
            pz = p_z.tile([H, BL], f32, tag="pz")
            pa = p_a.tile([H, BL], f32, tag="pa")
            pb = p_b.tile([H, BL], f32, tag="pb")
            one_r = r2 is None
            one_z = (r2 is None) and (mask_rhs is None)
            if rhs2 is xT:
                # encoder: x-side mms have no h-dependency -> emit first so
                # they prefetch on PE during the previous step's tail
                nc.tensor.matmul(pr[:], wc[:, wcol_r:wcol_r + H], r2,
                                 start=True, stop=False)
                nc.tensor.matmul(pz[:], wc[:, wcol_z:wcol_z + H], r2,
                                 start=True, stop=False)
                if mask_rhs is not None:
                    nc.tensor.matmul(pz[:], mrow[:], mask_rhs,
                                     start=False, stop=False)
                nc.tensor.matmul(pa[:], wc[:, wcol_a:wcol_a + H], r2,
                                 start=True, stop=True)
                nc.tensor.matmul(pr[:], wh[:, _HR:_HR + H], hB[:],
                                 start=False, stop=True)
                nc.tensor.matmul(pz[:], wh[:, _HZ:_HZ + H], hB[:],
                                 start=False, stop=True)
                nc.tensor.matmul(pb[:], wh[:, _HB:_HB + H], hB[:],
                                 start=True, stop=True)
                return pr, pz, pa, pb
            # r-group first and closed ASAP (tanh_r is on the critical chain)
            nc.tensor.matmul(pr[:], wh[:, (_FR if folded else _HR):][:, 0:H],
                             hB[:], start=True, stop=one_r)
            if r2 is not None:
                nc.tensor.matmul(pr[:], wc[:, wcol_r:wcol_r + H], r2,
                                 start=False, stop=True)
            nc.tensor.matmul(pz[:], wh[:, (_FZ if folded else _HZ):][:, 0:H],
                             hB[:], start=True, stop=one_z)
            if r2 is not None:
                nc.tensor.matmul(pz[:], wc[:, wcol_z:wcol_z + H], r2,
                                 start=False, stop=(mask_rhs is None))
            if mask_rhs is not None:
                nc.tensor.matmul(pz[:], mrow[:], mask_rhs, start=False, stop=True)
            nc.tensor.matmul(pb[:], wh[:, _HB:_HB + H], hB[:],
                             start=True, stop=True)
            if folded:
                nc.tensor.matmul(pa[:], wh[:, _FA2:_FA2 + H], hB[:],
                                 start=True, stop=one_r)
                if r2 is not None:
                    nc.tensor.matmul(pa[:], wc[:, wcol_a:wcol_a + H], r2,
                                     start=False, stop=True)
            else:
                nc.tensor.matmul(pa[:], wc[:, wcol_a:wcol_a + H], r2,
                                 start=True, stop=True)
            return pr, pz, pa, pb

        def gate_front(pr, pz, pa, pb, folded):
            """tanh_r/z + t2 + t3(into pb). Returns (tz, t3psum)."""
            tr = temps.tile([H, BL], bf, tag="tr")
            nc.scalar.activation(out=tr, in_=pr[:], func=AF.Tanh)
            tz = temps.tile([H, BL], bf, tag="tz")
            nc.scalar.activation(out=tz, in_=pz[:], func=AF.Tanh)
            t2 = temps.tile([H, BL], f32, tag="t2")
            if folded:
                nc.vector.tensor_mul(out=t2, in0=tr, in1=pb[:])
            else:
                nc.vector.scalar_tensor_tensor(out=t2, in0=tr, scalar=1.0,
                                               in1=pb[:], op0=OP.add, op1=OP.mult)
            nc.vector.tensor_add(out=pb[:], in0=t2, in1=pa[:])
            return tz

        def mix_tail(tz, pb_t3):
            """p1, tanh_n, pp, rr, h' -> hB."""
            p1 = temps.tile([H, BL], f32, tag="p1")
            nc.vector.scalar_tensor_tensor(out=p1, in0=tz, scalar=-0.5,
                                           in1=h64, op0=OP.mult, op1=OP.mult)
            n = temps.tile([H, BL], bf, tag="n")
            nc.scalar.activation(out=n, in_=pb_t3[:], func=AF.Tanh)
            pp = temps.tile([H, BL], f32, tag="pp")
            nc.vector.scalar_tensor_tensor(out=pp, in0=h64, scalar=0.5,
                                           in1=p1, op0=OP.mult, op1=OP.add)
            rr = temps.tile([H, BL], f32, tag="rr")
            nc.vector.scalar_tensor_tensor(out=rr, in0=tz, scalar=1.0,
                                           in1=n, op0=OP.add, op1=OP.mult)
            nc.vector.scalar_tensor_tensor(out=h64, in0=rr, scalar=0.5,
                                           in1=pp, op0=OP.mult, op1=OP.add)
            return n

        def emit_tail(t):
            """Attention tail + output for decoder step t (deferred emission)."""
            ly = p_ly.tile([BL, 141], f32, tag="ly")
            nc.tensor.matmul(ly[0:H, 0:128], wh[:, _WA:_WA + H], oB[:],
                             start=True, stop=True)
            nc.tensor.matmul(ly[0:BL, 128:141], oB[:], wh[:, _WF:_WF + OUT],
                             start=True, stop=True)
            e = temps.tile([H, BL], f32, tag="e")
            nc.scalar.activation(out=e, in_=ly[0:H, 0:128], func=AF.Exp)
            eo = temps.tile([H, BL], f32, tag="eo")
            nc.gpsimd.tensor_mul(out=eo, in0=e, in1=oB[0:H, :])
            nc.vector.tensor_add(out=sacc[:, 0:BL], in0=sacc[:, 0:BL], in1=e)
            nc.gpsimd.tensor_add(out=sacc[:, BL:2 * BL], in0=sacc[:, BL:2 * BL],
                                 in1=eo)
            return ly

        # ================= encoder =================
        for t in range(l_steps):
            mask_rhs = invm[:, t * BL:(t + 1) * BL] if t < l_steps - 1 else None
            pr, pz, pa, pb = gate_mms(_XR, _XZ, _XA, xT,
                                      slice(t * BL, (t + 1) * BL), mask_rhs, False)
            tz = gate_front(pr, pz, pa, pb, False)
            if t < l_steps - 1:
                mix_tail(tz, pb)
            else:
                n = temps.tile([H, BL], bf, tag="n")
                nc.scalar.activation(out=n, in_=pb[:], func=AF.Tanh)
                d = temps.tile([H, BL], f32, tag="d")
                nc.vector.tensor_sub(out=d, in0=n, in1=h64)
                tzd = temps.tile([H, BL], f32, tag="tzd")
                nc.vector.scalar_tensor_tensor(out=tzd, in0=tz, scalar=1.0,
                                               in1=d, op0=OP.add, op1=OP.mult)
                hn = temps.tile([H, BL], f32, tag="hn")
                nc.vector.scalar_tensor_tensor(out=hn, in0=tzd, scalar=0.5,
                                               in1=h64, op0=OP.mult, op1=OP.add)
                u = temps.tile([H, BL], f32, tag="u")
                nc.vector.tensor_mul(out=u, in0=m63, in1=tzd)
                nc.vector.scalar_tensor_tensor(out=h64, in0=u, scalar=0.5,
                                               in1=h64, op0=OP.mult, op1=OP.add)
                nc.vector.tensor_mul(out=oB[0:H, :], in0=hn, in1=m63)

        # ================= decoder (tail of step t-1 emitted inside step t) ===
        for t in range(t_steps):
            if t == 0:
                pr, pz, pa, pb = gate_mms(_CR, _CZ, _CA, oB, None, None, False)
            elif t == 1:
                pr, pz, pa, pb = gate_mms(_CR, _CZ, _CA, None, None, None, True)
            else:
                pr, pz, pa, pb = gate_mms(_CR, _CZ, _CA, attnB, None, None, True)
            tz = gate_front(pr, pz, pa, pb, folded=(t > 0))
            ly_prev = emit_tail(t - 1) if t > 0 else None
            mix_tail(tz, pb)
            if t == 0:
                nc.vector.tensor_copy(out=oB[0:H, :], in_=h64)
            else:
                rec = temps.tile([H, BL], f32, tag="rec")
                nc.vector.reciprocal_approx_fast(out=rec, in_=sacc[:, 0:BL])
                nc.vector.tensor_mul(out=attnB[0:H, :], in0=sacc[:, BL:2 * BL],
                                     in1=rec)
                nc.gpsimd.tensor_add(out=oB[0:H, :], in0=h64, in1=attnB[0:H, :])
            if ly_prev is not None:
                nc.scalar.copy(out=out_sb[:, (t - 1) * OUT:t * OUT],
                               in_=ly_prev[0:BL, 128:141])
        ly_last = emit_tail(t_steps - 1)
        nc.scalar.copy(out=out_sb[:, (t_steps - 1) * OUT:t_steps * OUT],
                       in_=ly_last[0:BL, 128:141])

        nc.sync.dma_start(out=d_out[:], in_=out_sb)
    if compile:
        nc.compile()
    return nc


def _make_in_maps(inputs, l_steps=L, t_steps=T):
    x = np.asarray(inputs["x"], np.float32)
    lengths = np.asarray(inputs["lengths"])
    w = _prep_weights(inputs["Wih"], inputs["Whh"], inputs["bih"],
                      inputs["bhh"], inputs["Wf"], inputs["bf"],
                      inputs["Wa"], inputs["ba"])
    in_maps = []
    for c in range(NCORES):
        sl = slice(c * BL, (c + 1) * BL)
        xT, invm, m63 = _prep_core(x[sl], lengths[sl], l_steps)
        in_maps.append(dict(xT=xT, invm=invm, m63=m63, **w))
    return in_maps


def kernel(**inputs):
    global LAST_EXEC_NS, TRACE_DIR
    from concourse.bass_utils import run_bass_kernel_spmd
    t_steps = int(inputs.get("output_length", T))
    assert t_steps == T, f"hardcoded for output_length={T}, got {t_steps}"
    nc = build_nc()
    in_maps = _make_in_maps(inputs)
    kw = {}
    if TRACE:
        import tempfile
        TRACE_DIR = tempfile.mkdtemp(prefix="bass_trace_")
        kw = dict(trace=True, tmpdir=TRACE_DIR)
    res = run_bass_kernel_spmd(nc, in_maps, list(range(NCORES)), **kw)
    LAST_EXEC_NS = res.exec_time_ns
    outs = [np.asarray(res.results[c]["out"]).reshape(BL, T, OUT)
            for c in range(NCORES)]
    return np.concatenate(outs, axis=0)
